# revision 44
# baseline (speedup 1.0000x reference)
"""Trainium2 Bass kernel for nn_EnergyDistributionCNN (3x3 conv -> unfold ->
softmax over patch -> weighted -> fold overlap-add), 8 NeuronCores.

Math (algebraically identical to the torch/jax reference):
    out = conv3x3(x, k)            cross-correlation, zero pad 1
    E   = exp(out)
    Z   = boxsum3x3(E padded with ONES)   (zero pads contribute exp(0)=1)
    U   = x / Z
    S   = boxsum3x3(U zero-padded)
    result = E * S

Sharding: row-block across 8 cores with a 3-row halo sliced on the host
(zero-filled at the global edges) -- no device-to-device communication.

All post-conv tensors are bf16 (host-measured error ~1.5e-2 max rel vs
the 2e-2 gate); conv stays fp32r (bf16 conv alone costs 1.8e-2). The
output is stored bf16 and upcast on the host, halving the out-DMA on the
serial DMA_ENGINES resource.

Engine split per width-half row-tile unit (~2050 cols):
  PE: conv as 3 shifted banded matmuls (fp32r); Z vertical band pass on
    hE (bf16, 1 pass) -- or, on z3-flagged units, 3 shifted BT passes on
    E directly (rebalances DVE->PE); S = 2 accumulating passes
    (BB@t_u + BB@U-shifted, bf16).
  Scalar: exp (masked via per-partition scale, bf16 out) and Copy
    (S PSUM -> bf16 SBUF drain). Both live in the same act table set.
  DVE: t_e = E + E(shift1) (bf16 2x), the fused custom op
    U = x * recip(Z) (quadratic-seed reciprocal + multiply in ONE
    8-stage DVE pass, reading Z straight from PSUM), t_u = U + U(shift1),
    and res = E * Sdrain (all-bf16 2x).
  GpSimd: hE = t_e + E(shift2) (the one wide op Pool can afford).

The custom DVE op RECIP_MUL_QUAD_ANT: 1/Z = bitcast(~Z) * p(t) with
t = Z*bitcast(~Z) in [-4.5, -4] (exponent-flip identity) and p a
degree-2 minimax fit of 1/t on that interval (rel err 5.1e-5), then * x.
Fits the 8-stage DVE ALU pipeline exactly; registered via the documented
dve_ops extension path.

Schedule: one emission iteration advances every unit's pipeline stage by
one. Within an iteration the PE stream interleaves chunk-wise
[conv_k(i), Zv_k(i-2), Smm_k(i-3)] so each cross-engine consumer (exp_k,
U_k, drain_k) finds its producer just-finished instead of parking; Z
PSUM chunks are consumed by the fused U op within the same iteration
(PSUM is only 8 banks). X tiles are DMA-prefetched one iteration ahead
so conv never parks on HBM; stores lag 5 iterations so SP's out-DMA
issue never head-of-line-blocks the X prefetch stream.
"""

from contextlib import ExitStack

import numpy as np

import concourse.bacc as bacc
import concourse.mybir as mybir
import concourse.tile as tile
from concourse._compat import with_exitstack
from concourse.bass_utils import run_bass_kernel_spmd

F32 = mybir.dt.float32
F32R = mybir.dt.float32r
BF16 = mybir.dt.bfloat16

H = 4096
W = 4096
N_CORES = 8
RC = H // N_CORES  # rows per core
HALO = 3
RT = 122   # output rows per row-tile (RT + 6 <= 128 partitions)
WS = 2     # width splits (SBUF capacity)
WH = W // WS
MM = 512   # matmul moving-operand max free size / one fp32 PSUM bank
USE_CUSTOM_U = True

# quadratic minimax fit of 1/t on t in [-4.5, -4] (rel err 5.1e-5)
RQ_C0 = -0.7071054765951768
RQ_C1 = -0.16652166157425166
RQ_C2 = -0.013060520969582767


# ----------------------------------------------------- custom DVE op (fused)

_RECIP_MUL = None


def _register_recip_mul():
    """U = in1 * (1/in0) in one DVE pass: exponent-flip seed + quadratic
    polish + multiply. Registered through the documented dve_ops extension
    path (OPS append + sub-opcode row); sha computed at registration."""
    global _RECIP_MUL
    if _RECIP_MUL is not None:
        return _RECIP_MUL
    from concourse import dve_ops
    from concourse.dve_spec import AluOp, Bin, Spec, Src0, Src1, C0, C1, C2, lower
    from concourse.dve_uop import DveOpSpec

    name = "RECIP_MUL_QUAD_ANT"
    if name in dve_ops._SUB_OPCODE_FOR_NAME:
        _RECIP_MUL = next(op for op in dve_ops.OPS if op.name == name)
        return _RECIP_MUL

    _not = Bin(AluOp.BITWISE_NOT, Src0, Src0)
    _t = Src0 * _not
    body = ((_t * C2 + C1) * _t + C0) * _not * Src1

    def ref(in0, in1, c0, c1, c2):
        z = np.ascontiguousarray(in0, np.float32)
        nx = (~z.view(np.int32)).view(np.float32)
        t = z * nx
        return ((t * c2 + c1) * t + c0) * nx * np.asarray(in1, np.float32)

    spec = Spec(body=body, reference=ref)
    row = max(dve_ops._SUB_OPCODE_FOR_NAME.values()) + 1
    assert row < 0x20, "custom-DVE row field overflow"
    dve_ops._SUB_OPCODE_FOR_NAME[name] = row
    shas = {}
    for ver in ("v3", "v4"):
        uops = lower(spec, ver=ver)
        shas[ver] = DveOpSpec(name=name, opcode=row, uops=uops, rd1_en=True).sha(ver)
    op = dve_ops.DveOp(name, spec, subdim=False, uops_sha=shas)
    dve_ops.OPS.append(op)
    dve_ops.CUSTOM_DVE_SPECS[name] = spec
    _RECIP_MUL = op
    return op


# ---------------------------------------------------------------- host side

def _make_bands(k: np.ndarray) -> np.ndarray:
    """bands[v][p, m] = k[p-m, v] (conv, v=0..2); bands[3] = BB ones with
    p-m in 0..2 (S matmul); bands[4] = BT ones with m-p in 0..2 (Z).
    bands[5..9]: same five patterns as 4x block-diagonal 32x32 blocks, for
    the column-folded last row-tile."""
    bands = np.zeros((10, 128, 128), np.float32)
    idx = np.arange(128)
    for d in range(3):
        p = idx[d:]
        m = idx[: 128 - d]
        for v in range(3):
            bands[v, p, m] = k[d, v]
        bands[3, p, m] = 1.0
        bands[4, m, p] = 1.0
    for i in range(5):
        blk = bands[i][:32, :32]
        for b in range(4):
            bands[5 + i][32 * b : 32 * b + 32, 32 * b : 32 * b + 32] = blk
    return bands


def _make_core_inputs(x: np.ndarray, bands: np.ndarray, core: int):
    r0 = core * RC
    lo, hi = r0 - HALO, r0 + RC + HALO
    # 26 extra zero rows let the folded last tile load full 32-row blocks
    xh = np.zeros((RC + 2 * HALO + 26, W + 2 * HALO), np.float32)
    s_lo, s_hi = max(lo, 0), min(hi, H)
    xh[s_lo - lo : s_hi - lo, HALO : HALO + W] = x[s_lo:s_hi]
    gl = np.arange(lo, hi)
    mask = ((gl >= 0) & (gl < H)).astype(np.float32)
    # pre-tiled per-row-tile mask: column j = exp-scale rows for tile j
    # (rows o+1 .. o+R+4); the fold tile's column is laid out in its
    # 4x32-partition block structure with zeros on the unused lanes.
    tiles = _make_tiles()
    mk = np.zeros((128, len(tiles)), np.float32)
    for j, (o, R) in enumerate(tiles[:-1]):
        mk[: R + 4, j] = mask[o + 1 : o + R + 5]
    of, Rf = tiles[-1]
    if Rf <= 26:
        for b in range(4):
            mk[32 * b : 32 * b + Rf + 4, len(tiles) - 1] = mask[of + 1 : of + Rf + 5]
    else:
        mk[: Rf + 4, len(tiles) - 1] = mask[of + 1 : of + Rf + 5]
    return {"xh": xh, "mask": mk, "bands": bands}


def _make_tiles():
    tiles = []
    o = 0
    while o < RC:
        R = min(RT, RC - o)
        tiles.append((o, R))
        o += R
    return tiles


def _chunks(total: int, step: int = MM):
    out = []
    s = 0
    while s < total:
        out.append((s, min(step, total - s)))
        s += step
    return out


# -------------------------------------------------------------- device side

@with_exitstack
def _energy_body(ctx: ExitStack, tc, out_d, out2_d, xh_d, mask_d, bands_d, dbg=None):
    nc = tc.nc
    Exp = mybir.ActivationFunctionType.Exp
    Cpy = mybir.ActivationFunctionType.Copy
    rm_op = _register_recip_mul()
    Cpy2 = mybir.ActivationFunctionType.Copy

    # ---- constants. Conv bands are used directly as fp32r bitcast views;
    # BT/BB (ones bands) additionally as bf16 for the bf16 moving operands.
    consts = ctx.enter_context(tc.tile_pool(name="consts", bufs=1))
    scratch = consts.tile([1, 2], F32, name="scratch")
    nc.vector.memset(scratch, 0.0)
    # dummy activation at t=0 hoists the 1283ns Exp-table load off the
    # critical path (it would otherwise sit behind the first exp's waits)
    nc.scalar.activation(scratch[:, 0:1], scratch[:, 1:2],
                         mybir.ActivationFunctionType.Exp)
    warm = consts.tile([128, 128], F32R, name="warm")
    nc.vector.memset(warm.bitcast(F32), 0.0)
    bigb = consts.tile([128, 10 * 128], F32R, name="bigb")
    nc.sync.dma_start(
        out=bigb[:, 5 * 128 :].rearrange("p (i m) -> p i m", i=5),
        in_=bands_d[5:].rearrange("i p m -> p i m"),
    )

    def band(i):
        return bigb[:, i * 128 : (i + 1) * 128]

    MB = [band(v) for v in range(3)]       # conv bands, normal
    MBF = [band(5 + v) for v in range(3)]  # conv bands, folded
    BBb = consts.tile([128, 128], BF16, name="bbb")    # S band bf16, normal
    BTb = consts.tile([128, 128], BF16, name="btb")    # Z band bf16, normal
    BBFb = consts.tile([128, 128], BF16, name="bbfb")  # S band bf16, folded
    BTFb = consts.tile([128, 128], BF16, name="btfb")  # Z band bf16, folded
    nc.vector.tensor_copy(out=BBFb, in_=bigb[:, 8 * 128 : 9 * 128].bitcast(F32))
    nc.vector.tensor_copy(out=BTFb, in_=bigb[:, 9 * 128 : 10 * 128].bitcast(F32))

    def load_conv_bands():
        # deferred until after the first (folded) unit's X DMAs so the
        # pipeline-fill unit's inputs are first in the DMA queue
        nc.sync.dma_start(
            out=bigb[:, : 3 * 128].rearrange("p (i m) -> p i m", i=3),
            in_=bands_d[:3].rearrange("i p m -> p i m"),
        )

    def load_sum_bands():
        nc.sync.dma_start(
            out=bigb[:, 3 * 128 : 5 * 128].rearrange("p (i m) -> p i m", i=2),
            in_=bands_d[3:5].rearrange("i p m -> p i m"),
        )
        nc.vector.tensor_copy(out=BBb, in_=bigb[:, 3 * 128 : 4 * 128].bitcast(F32))
        nc.vector.tensor_copy(out=BTb, in_=bigb[:, 4 * 128 : 5 * 128].bitcast(F32))

    SEGW = WH // 4
    tiles = _make_tiles()
    RES_POOL_UNITS = {2, 4, 6, 8}

    # all row-tile exp-scale masks arrive in one small DMA (host pre-tiled)
    mk_all = consts.tile([128, len(tiles)], F32, name="mk_all")
    nc.sync.dma_start(out=mk_all, in_=mask_d)

    xpool = ctx.enter_context(tc.tile_pool(name="xp", bufs=7))
    epool = ctx.enter_context(tc.tile_pool(name="ep", bufs=7))
    tepool = ctx.enter_context(tc.tile_pool(name="tep", bufs=2))
    hepool = ctx.enter_context(tc.tile_pool(name="hep", bufs=3))
    upool = ctx.enter_context(tc.tile_pool(name="up", bufs=3))
    tupool = ctx.enter_context(tc.tile_pool(name="tup", bufs=3))
    sbpool = ctx.enter_context(tc.tile_pool(name="sbp", bufs=3))
    zspool = ctx.enter_context(tc.tile_pool(name="zsp", bufs=2))
    respool = ctx.enter_context(tc.tile_pool(name="resp", bufs=4))
    ps_conv = ctx.enter_context(tc.tile_pool(name="psc", bufs=2, space="PSUM"))
    ps_z = ctx.enter_context(tc.tile_pool(name="psz", bufs=2, space="PSUM"))
    ps_s = ctx.enter_context(tc.tile_pool(name="pss", bufs=2, space="PSUM"))

    _prep_count = [0]

    def prep(unit, q=None):
        """Allocate per-unit tiles + DMA X (called one iteration ahead).
        q overrides the DMA issue queue (Act for the first loads: overlaps
        SP's issue latency during fill; Act is idle until the first exp)."""
        o, R, g0, cw, fold, tj, z3 = unit
        q = q or nc.sync
        _prep_count[0] += 1
        EW = (SEGW if fold else cw) + 4   # E width
        UW = EW - 2                       # U / Rz width
        X = xpool.tile([128, EW + 2], F32R, tag="X")
        if fold:
            for b in range(4):
                (q if b % 2 else nc.sync).dma_start(
                    out=X[32 * b : 32 * b + 32, :],
                    in_=xh_d[o : o + 32, g0 + b * SEGW : g0 + b * SEGW + SEGW + 6],
                )
        else:
            half = (cw + 6) // 2
            nc.sync.dma_start(
                out=X[: R + 6, :half], in_=xh_d[o : o + R + 6, g0 : g0 + half]
            )
            q.dma_start(
                out=X[: R + 6, half : cw + 6],
                in_=xh_d[o : o + R + 6, g0 + half : g0 + cw + 6],
            )
        return dict(
            o=o, R=R, g0=g0, cw=cw, fold=fold, tj=tj, z3=z3, EW=EW, UW=UW,
            res_pool=(not fold) and (tj != len(tiles) - 1)
            and (_prep_count[0] % 2 == 0),
            X=X,
            rows_in=slice(0, 128) if fold else slice(0, R + 6),
            rows_e=slice(0, 128) if fold else slice(0, R + 4),
            rows_s=slice(0, 128) if fold else slice(0, R + 2),
            OW=SEGW if fold else cw,
        )

    def emit_conv_chunk(st, cs, cl):
        """conv chunk (PE x3 per 512 sub-chunk, fp32r) into a 1024-wide
        (2-bank) PSUM tile -> one wide exp per tile (Act, bf16 out)."""
        rows_in, rows_e = st["rows_in"], st["rows_e"]
        mb = MBF if st["fold"] else MB
        mk = mk_all[:, st["tj"] : st["tj"] + 1]
        pc = ps_conv.tile([128, cl], F32, tag="pc", name="pc")
        for bs, bl in _chunks(cl):
            for v in range(3):
                nc.tensor.matmul(
                    pc[rows_e, bs : bs + bl],
                    mb[v][rows_in, rows_e],
                    st["X"][rows_in, cs + bs + v : cs + bs + v + bl],
                    start=(v == 0),
                    stop=(v == 2),
                )
        nc.scalar.activation(
            st["E"][rows_e, cs : cs + cl], pc[rows_e, :cl], Exp,
            scale=mk if st["fold"] else mk[: st["R"] + 4],
        )

    def stage_hsum(st):
        """Edge-pad memsets (must precede t_e) + Zh off-PE path:
        t_e = E + E(shift1) (DVE bf16 2x), hE = t_e + E(shift2) (GpSimd).
        z3 units only get the memsets."""
        rows_e, EW, UW, E = st["rows_e"], st["EW"], st["UW"], st["E"]
        # E at global-edge pad columns must be exp(0)=1: the conv window
        # at pad col -1 / W overlaps one real column, so it is NOT zero
        if st["g0"] == 0:
            er = slice(0, 32) if st["fold"] else rows_e
            nc.vector.memset(E[er, 0:2], 1.0)
        if st["g0"] + st["cw"] == W:
            er = slice(96, 128) if st["fold"] else rows_e
            nc.vector.memset(E[er, EW - 2 : EW], 1.0)
        if st["z3"]:
            return
        t_e = tepool.tile([128, EW - 1], BF16, tag="te")
        nc.vector.tensor_add(
            out=t_e[rows_e, :], in0=E[rows_e, : EW - 1], in1=E[rows_e, 1:EW]
        )
        hE = hepool.tile([128, UW], BF16, tag="hE")
        nc.gpsimd.tensor_add(
            out=hE[rows_e, :], in0=t_e[rows_e, :UW], in1=E[rows_e, 2:EW]
        )
        st["hE"] = hE

    def emit_z_chunk(st, cs, cl):
        """Z vertical pass chunk (PE, per 512-wide bank) into a 1024-wide
        (2-bank) PSUM tile -> ONE fused U chunk per tile (DVE custom op,
        Z straight from PSUM, bf16 out): halves the DVE PSUM-access
        per-instruction overhead. Consumed in-iteration: PSUM is 8 banks."""
        rows_e = st["rows_e"]
        bt = BTFb if st["fold"] else BTb
        pz = ps_z.tile([128, cl], F32, tag="pz", name="pz")
        for bs, bl in _chunks(cl):
            if st["z3"]:
                for v in range(3):
                    nc.tensor.matmul(
                        pz[rows_e, bs : bs + bl],
                        bt[rows_e, rows_e],
                        st["E"][rows_e, cs + bs + v : cs + bs + v + bl],
                        start=(v == 0),
                        stop=(v == 2),
                    )
            else:
                nc.tensor.matmul(
                    pz[rows_e, bs : bs + bl],
                    bt[rows_e, rows_e],
                    st["hE"][rows_e, cs + bs : cs + bs + bl],
                    start=True,
                    stop=True,
                )
        if USE_CUSTOM_U:
            nc.vector._custom_dve(
                rm_op,
                out=st["U"][rows_e, cs : cs + cl],
                in0=pz[rows_e, :cl],
                in1=st["X"].bitcast(F32)[rows_e, 2 + cs : 2 + cs + cl],
                s0=RQ_C0, s1=RQ_C1, imm2=RQ_C2,
            )
        else:
            Zs = st["Zs"]
            nc.scalar.activation(Zs[rows_e, cs : cs + cl], pz[rows_e, :cl], Cpy2)
            nc.vector.reciprocal_approx_fast(
                out=Zs[rows_e, cs : cs + cl], in_=Zs[rows_e, cs : cs + cl]
            )
            nc.gpsimd.tensor_mul(
                out=st["U"][rows_e, cs : cs + cl],
                in0=st["X"].bitcast(F32)[rows_e, 2 + cs : 2 + cs + cl],
                in1=Zs[rows_e, cs : cs + cl],
            )

    def stage_tu(st):
        """t_u = U + U(shift1), bf16. DVE (2x) for hE-path units; GpSimd
        for z3 units (whose Pool is otherwise idle) to unload the DVE."""
        rows_e, UW, U = st["rows_e"], st["UW"], st["U"]
        t_u = tupool.tile([128, UW - 1], BF16, tag="tu")
        eng = nc.vector
        eng.tensor_add(
            out=t_u[rows_e, :], in0=U[rows_e, : UW - 1], in1=U[rows_e, 1:UW]
        )
        st["t_u"] = t_u

    def emit_s_chunk(st, cs, cl):
        """S chunk = BB@t_u + BB@U(shift2) (PE bf16) -> Act drains the
        S PSUM chunk to bf16 SBUF."""
        rows_e, rows_s = st["rows_e"], st["rows_s"]
        bb = BBFb if st["fold"] else BBb
        ps = ps_s.tile([128, MM], F32, tag="ps", name="ps")
        nc.tensor.matmul(
            ps[rows_s, :cl], bb[rows_e, rows_s],
            st["t_u"][rows_e, cs : cs + cl],
            start=True, stop=False,
        )
        nc.tensor.matmul(
            ps[rows_s, :cl], bb[rows_e, rows_s],
            st["U"][rows_e, cs + 2 : cs + 2 + cl],
            start=False, stop=True,
        )
        nc.scalar.activation(st["Sb"][rows_s, cs : cs + cl], ps[rows_s, :cl], Cpy)

    def stage_res(st):
        """res = E * Sdrain (DVE), one wide op. The last row-tile writes
        f32 (its bf16 store corrupts on HW); folds go per-32-block so each
        block's store can issue while the next block's res computes."""
        rows_s, OW = st["rows_s"], st["OW"]
        lastt = st["tj"] == len(tiles) - 1
        res = respool.tile(
            [128, OW], F32 if lastt else BF16,
            tag="resf" if lastt else "res", name="res",
        )
        if st["fold"]:
            for b in range(4):
                rs = slice(32 * b, 32 * b + st["R"] + 4)
                eng = nc.gpsimd if b % 2 else nc.vector
                eng.tensor_mul(
                    out=res[rs, :OW],
                    in0=st["E"][rs, 2 : 2 + OW],
                    in1=st["Sb"][rs, :OW],
                )
        else:
            # alternating units multiply on GpSimd: Pool has slack and this
            # op is emitted after hE, so it never delays hE (PE's Zv input)
            eng = nc.gpsimd if st.get("res_pool") else nc.vector
            eng.tensor_mul(
                out=res[rows_s, :OW],
                in0=st["E"][rows_s, 2 : 2 + OW],
                in1=st["Sb"][rows_s, :OW],
            )
        st["res"] = res

    def store(st):
        # lagged well behind stage_res so SP's out-DMA issue rarely waits
        # on an unfinished res (which would head-of-line-block the next
        # X prefetch in the queue). The last row-tile goes to the separate
        # f32 out2 tensor: bf16 stores from the small-R tile corrupt even
        # columns on real HW (CoreSim clean); f32 stores never did.
        o, R, g0, fold, res = st["o"], st["R"], st["g0"], st["fold"], st["res"]
        lastt = st["tj"] == len(tiles) - 1
        dst = out2_d if lastt else out_d
        ro = o - tiles[-1][0] if lastt else o
        if fold:
            qs = [nc.sync, nc.sync, nc.sync, nc.sync]
            for b in range(4):
                qs[b].dma_start(
                    out=dst[ro : ro + R, g0 + b * SEGW : g0 + (b + 1) * SEGW],
                    in_=res[32 * b + 2 : 32 * b + 2 + R, :SEGW],
                )
        else:
            nc.sync.dma_start(
                out=dst[ro : ro + R, g0 : g0 + st["cw"]], in_=res[2 : R + 2, :st["cw"]]
            )

    of, Rf = tiles[-1]
    units = []
    if len(tiles) > 1 and Rf <= 26:
        # Both folded units lead: their X DMAs are tiny (32-row blocks) so
        # the engines saturate immediately while the serial DMA queue
        # streams the f32 X tiles of the wide units. Their f32 stores also
        # leave the tail to the two half-width units (short drain chains).
        # z3 flags: folds + every 4th normal unit use the 3-pass Z (PE)
        # instead of t_e/hE (DVE+Pool) -- balances PE vs DVE load.
        units.append((of, Rf, 0, WH, True, len(tiles) - 1, True))
        nrm = []
        for j, (o, R) in enumerate(tiles[:-1]):
            for h in range(WS):
                nrm.append((o, R, h * WH, WH, False, j))
        # first normal unit split in two: halves the X DMA the fill waits on
        o0, R0, g00, cw0, f0, j0 = nrm[0]
        units.append((o0, R0, g00, cw0 // 2, f0, j0, False))
        units.append((o0, R0, g00 + cw0 // 2, cw0 // 2, f0, j0, False))
        for i, (o, R, g0, cw, fold, j) in enumerate(nrm[1:-1]):
            units.append((o, R, g0, cw, fold, j, i % 4 == 1))
        # (res_pool flags are set on states in prep below)
        # split the trailing normal unit in two: at drain time only the
        # cheap fold remains to hide a unit's cross-engine chain
        o, R, g0, cw, fold, j = nrm[-1]
        units.append((o, R, g0, cw // 2, fold, j, False))
        units.append((o, R, g0 + cw // 2, cw // 2, fold, j, False))
        units.append((of, Rf, WH, WH, True, len(tiles) - 1, True))
    else:
        for j, (o, R) in enumerate(tiles):
            for h in range(WS):
                units.append((o, R, h * WH, WH, False, j, h == 0))
    n_real = len(units)

    states = [None] * len(units)
    n = len(units)

    def is_dummy(idx):
        return idx >= n_real
    # ~3.5us of dummy matmuls while the first DMAs land: the PE p-state
    # ramps to full clock only after 3us of continuous work, so the first
    # real conv then runs at 2.4GHz instead of half speed
    for _ in range(18):
        pw = ps_conv.tile([128, MM], F32, tag="pc")
        nc.tensor.matmul(pw[:, :128], warm, warm, start=True, stop=True)

    states[0] = prep(units[0])
    load_conv_bands()
    if n > 1:
        states[1] = prep(units[1])
    load_sum_bands()
    for i in range(n + 6):
        # X prefetch two iterations ahead of conv
        if i + 2 < n:
            states[i + 2] = prep(units[i + 2])
        st_c = states[i] if i < n else None
        st_h = states[i - 1] if 1 <= i <= n else None
        st_z = states[i - 3] if 3 <= i <= n + 2 else None
        st_s = states[i - 4] if 4 <= i <= n + 3 else None

        if st_c is not None:
            st_c["E"] = epool.tile([128, st_c["EW"]], BF16, tag="E", name="E")
        if st_z is not None:
            st_z["U"] = upool.tile([128, st_z["UW"]], BF16, tag="U", name="U")
            if not USE_CUSTOM_U:
                st_z["Zs"] = zspool.tile([128, st_z["UW"]], F32, tag="Zs", name="Zs")
        if st_s is not None:
            st_s["Sb"] = sbpool.tile([128, st_s["OW"]], BF16, tag="Sb", name="Sb")

        # DVE queue first: edge memsets + t_e of unit i-1 (their inputs
        # finished last iteration), then Pool's hE
        if st_h is not None:
            stage_hsum(st_h)

        # PE stream interleaved chunk-wise so Act/DVE consumers never park
        cc = _chunks(st_c["EW"]) if st_c is not None else []
        zc = _chunks(st_z["UW"], 1024) if st_z is not None else []
        sc = _chunks(st_s["OW"]) if st_s is not None else []
        for k in range(max(len(cc), len(zc), len(sc))):
            if k < len(cc):
                emit_conv_chunk(st_c, *cc[k])
            if k < len(zc):
                emit_z_chunk(st_z, *zc[k])
            if k < len(sc):
                emit_s_chunk(st_s, *sc[k])

        if st_z is not None:
            stage_tu(st_z)
        if 5 <= i <= n + 4:
            stage_res(states[i - 5])
        if 6 <= i <= n + 5:
            store(states[i - 6])

    if dbg is not None:
        st = states[n_real - 1]
        for key in dbg:
            if key == "ps":
                psf = respool.tile([128, MM], F32, name="psf", tag="psf")
                nc.scalar.activation(psf, st["ps_dbg"], Cpy)
                nc.sync.dma_start(out=dbg[key], in_=psf)
            else:
                src_t = st[{"E": "E", "U": "U", "Sb": "Sb", "res": "res", "tu": "t_u"}[key]]
                nc.sync.dma_start(out=dbg[key], in_=src_t)


_CACHE: dict = {}


def _build(dbg_mode=False):
    key = ("nc_dbg" if dbg_mode else "nc")
    if key in _CACHE:
        return _CACHE[key]
    nc = bacc.Bacc(
        "TRN2", target_bir_lowering=False, debug=False, num_devices=N_CORES
    )
    xh_d = nc.dram_tensor(
        "xh", (RC + 2 * HALO + 26, W + 2 * HALO), F32R, kind="ExternalInput"
    ).ap()
    mask_d = nc.dram_tensor(
        "mask", (128, len(_make_tiles())), F32, kind="ExternalInput"
    ).ap()
    bands_d = nc.dram_tensor("bands", (10, 128, 128), F32R, kind="ExternalInput").ap()
    out_d = nc.dram_tensor("out", (RC, W), BF16, kind="ExternalOutput").ap()
    out2_d = nc.dram_tensor(
        "out2", (RC - _make_tiles()[-1][0], W), F32, kind="ExternalOutput"
    ).ap()
    dbg = None
    if dbg_mode:
        SEGW = WH // 4
        dbg = {
            "E": nc.dram_tensor("dbgE", (128, SEGW + 4), BF16, kind="ExternalOutput").ap(),
            "U": nc.dram_tensor("dbgU", (128, SEGW + 2), BF16, kind="ExternalOutput").ap(),
            "Sb": nc.dram_tensor("dbgSb", (128, SEGW), BF16, kind="ExternalOutput").ap(),
            "res": nc.dram_tensor("dbgres", (128, SEGW), BF16, kind="ExternalOutput").ap(),
            "tu": nc.dram_tensor("dbgtu", (128, SEGW + 1), BF16, kind="ExternalOutput").ap(),
        }
    with tile.TileContext(nc) as tc:
        _energy_body(tc, out_d, out2_d, xh_d, mask_d, bands_d, dbg=dbg)
    nc.compile()
    _CACHE[key] = nc
    return nc


def kernel(shareable_energy: np.ndarray, kernel: np.ndarray, **_run_kw) -> np.ndarray:
    x = np.ascontiguousarray(np.asarray(shareable_energy, np.float32))
    k = np.asarray(kernel, np.float32)
    assert x.shape == (H, W), x.shape
    nc = _build()
    bands = _make_bands(k)
    in_maps = [_make_core_inputs(x, bands, core) for core in range(N_CORES)]
    r = run_bass_kernel_spmd(nc, in_maps, core_ids=list(range(N_CORES)), **_run_kw)
    o_last = _make_tiles()[-1][0]
    out = np.concatenate(
        [
            np.concatenate(
                [
                    np.asarray(res["out"]).astype(np.float32)[:o_last],
                    np.asarray(res["out2"]),
                ],
                axis=0,
            )
            for res in r.results
        ],
        axis=0,
    )
    if _run_kw:
        _CACHE["last_result"] = r
    return out


# revision 46
# speedup vs baseline: 1.0243x; 1.0243x over previous
"""Trainium2 Bass kernel for nn_EnergyDistributionCNN (3x3 conv -> unfold ->
softmax over patch -> weighted -> fold overlap-add), 8 NeuronCores.

Math (algebraically identical to the torch/jax reference):
    out = conv3x3(x, k)            cross-correlation, zero pad 1
    E   = exp(out)
    Z   = boxsum3x3(E padded with ONES)   (zero pads contribute exp(0)=1)
    U   = x / Z
    S   = boxsum3x3(U zero-padded)
    result = E * S

Sharding: row-block across 8 cores with a 3-row halo sliced on the host
(zero-filled at the global edges) -- no device-to-device communication.

All post-conv tensors are bf16 (host-measured error ~1.5e-2 max rel vs
the 2e-2 gate); conv stays fp32r (bf16 conv alone costs 1.8e-2). The
output is stored bf16 and upcast on the host, halving the out-DMA on the
serial DMA_ENGINES resource.

Engine split per width-half row-tile unit (~2050 cols):
  PE: conv as 3 shifted banded matmuls (fp32r); Z vertical band pass on
    hE (bf16, 1 pass) -- or, on z3-flagged units, 3 shifted BT passes on
    E directly (rebalances DVE->PE); S = 2 accumulating passes
    (BB@t_u + BB@U-shifted, bf16).
  Scalar: exp (masked via per-partition scale, bf16 out) and Copy
    (S PSUM -> bf16 SBUF drain). Both live in the same act table set.
  DVE: t_e = E + E(shift1) (bf16 2x), the fused custom op
    U = x * recip(Z) (quadratic-seed reciprocal + multiply in ONE
    8-stage DVE pass, reading Z straight from PSUM), t_u = U + U(shift1),
    and res = E * Sdrain (all-bf16 2x).
  GpSimd: hE = t_e + E(shift2) (the one wide op Pool can afford).

The custom DVE op RECIP_MUL_QUAD_ANT: 1/Z = bitcast(~Z) * p(t) with
t = Z*bitcast(~Z) in [-4.5, -4] (exponent-flip identity) and p a
degree-2 minimax fit of 1/t on that interval (rel err 5.1e-5), then * x.
Fits the 8-stage DVE ALU pipeline exactly; registered via the documented
dve_ops extension path.

Schedule: one emission iteration advances every unit's pipeline stage by
one. Within an iteration the PE stream interleaves chunk-wise
[conv_k(i), Zv_k(i-2), Smm_k(i-3)] so each cross-engine consumer (exp_k,
U_k, drain_k) finds its producer just-finished instead of parking; Z
PSUM chunks are consumed by the fused U op within the same iteration
(PSUM is only 8 banks). X tiles are DMA-prefetched one iteration ahead
so conv never parks on HBM; stores lag 5 iterations so SP's out-DMA
issue never head-of-line-blocks the X prefetch stream.
"""

from contextlib import ExitStack

import numpy as np

import concourse.bacc as bacc
import concourse.mybir as mybir
import concourse.tile as tile
from concourse._compat import with_exitstack
from concourse.bass_utils import run_bass_kernel_spmd

F32 = mybir.dt.float32
F32R = mybir.dt.float32r
BF16 = mybir.dt.bfloat16

H = 4096
W = 4096
N_CORES = 8
RC = H // N_CORES  # rows per core
HALO = 3
RT = 122   # output rows per row-tile (RT + 6 <= 128 partitions)
WS = 2     # width splits (SBUF capacity)
WH = W // WS
MM = 512   # matmul moving-operand max free size / one fp32 PSUM bank
USE_CUSTOM_U = True

# quadratic minimax fit of 1/t on t in [-4.5, -4] (rel err 5.1e-5)
RQ_C0 = -0.7071054765951768
RQ_C1 = -0.16652166157425166
RQ_C2 = -0.013060520969582767


# ----------------------------------------------------- custom DVE op (fused)

_RECIP_MUL = None


def _register_recip_mul():
    """U = in1 * (1/in0) in one DVE pass: exponent-flip seed + quadratic
    polish + multiply. Registered through the documented dve_ops extension
    path (OPS append + sub-opcode row); sha computed at registration."""
    global _RECIP_MUL
    if _RECIP_MUL is not None:
        return _RECIP_MUL
    from concourse import dve_ops
    from concourse.dve_spec import AluOp, Bin, Spec, Src0, Src1, C0, C1, C2, lower
    from concourse.dve_uop import DveOpSpec

    name = "RECIP_MUL_QUAD_ANT"
    if name in dve_ops._SUB_OPCODE_FOR_NAME:
        _RECIP_MUL = next(op for op in dve_ops.OPS if op.name == name)
        return _RECIP_MUL

    _not = Bin(AluOp.BITWISE_NOT, Src0, Src0)
    _t = Src0 * _not
    body = ((_t * C2 + C1) * _t + C0) * _not * Src1

    def ref(in0, in1, c0, c1, c2):
        z = np.ascontiguousarray(in0, np.float32)
        nx = (~z.view(np.int32)).view(np.float32)
        t = z * nx
        return ((t * c2 + c1) * t + c0) * nx * np.asarray(in1, np.float32)

    spec = Spec(body=body, reference=ref)
    row = max(dve_ops._SUB_OPCODE_FOR_NAME.values()) + 1
    assert row < 0x20, "custom-DVE row field overflow"
    dve_ops._SUB_OPCODE_FOR_NAME[name] = row
    shas = {}
    for ver in ("v3", "v4"):
        uops = lower(spec, ver=ver)
        shas[ver] = DveOpSpec(name=name, opcode=row, uops=uops, rd1_en=True).sha(ver)
    op = dve_ops.DveOp(name, spec, subdim=False, uops_sha=shas)
    dve_ops.OPS.append(op)
    dve_ops.CUSTOM_DVE_SPECS[name] = spec
    _RECIP_MUL = op
    return op


# ---------------------------------------------------------------- host side

def _make_bands(k: np.ndarray) -> np.ndarray:
    """bands[v][p, m] = k[p-m, v] (conv, v=0..2); bands[3] = BB ones with
    p-m in 0..2 (S matmul); bands[4] = BT ones with m-p in 0..2 (Z).
    bands[5..9]: same five patterns as 4x block-diagonal 32x32 blocks, for
    the column-folded last row-tile."""
    bands = np.zeros((10, 128, 128), np.float32)
    idx = np.arange(128)
    for d in range(3):
        p = idx[d:]
        m = idx[: 128 - d]
        for v in range(3):
            bands[v, p, m] = k[d, v]
        bands[3, p, m] = 1.0
        bands[4, m, p] = 1.0
    for i in range(5):
        blk = bands[i][:32, :32]
        for b in range(4):
            bands[5 + i][32 * b : 32 * b + 32, 32 * b : 32 * b + 32] = blk
    return bands


def _make_core_inputs(x: np.ndarray, bands: np.ndarray, core: int):
    r0 = core * RC
    lo, hi = r0 - HALO, r0 + RC + HALO
    # 26 extra zero rows let the folded last tile load full 32-row blocks
    xh = np.zeros((RC + 2 * HALO + 26, W + 2 * HALO), np.float32)
    s_lo, s_hi = max(lo, 0), min(hi, H)
    xh[s_lo - lo : s_hi - lo, HALO : HALO + W] = x[s_lo:s_hi]
    gl = np.arange(lo, hi)
    mask = ((gl >= 0) & (gl < H)).astype(np.float32)
    # fold-unit X tiles pre-packed: 4 column blocks stacked in partitions,
    # so each fold unit's X arrives in ONE DMA instead of four
    tiles0 = _make_tiles()
    of0, _Rf0 = tiles0[-1]
    SEGW = (W // WS) // 4
    xf = np.zeros((WS, 128, SEGW + 6), np.float32)
    for u in range(WS):
        for b in range(4):
            c0 = u * (W // WS) + b * SEGW
            xf[u, 32 * b : 32 * b + 32, :] = xh[of0 : of0 + 32, c0 : c0 + SEGW + 6]
    # pre-tiled per-row-tile mask: column j = exp-scale rows for tile j
    # (rows o+1 .. o+R+4); the fold tile's column is laid out in its
    # 4x32-partition block structure with zeros on the unused lanes.
    tiles = _make_tiles()
    mk = np.zeros((128, len(tiles)), np.float32)
    for j, (o, R) in enumerate(tiles[:-1]):
        mk[: R + 4, j] = mask[o + 1 : o + R + 5]
    of, Rf = tiles[-1]
    if Rf <= 26:
        for b in range(4):
            mk[32 * b : 32 * b + Rf + 4, len(tiles) - 1] = mask[of + 1 : of + Rf + 5]
    else:
        mk[: Rf + 4, len(tiles) - 1] = mask[of + 1 : of + Rf + 5]
    return {"xh": xh, "mask": mk, "bands": bands, "xf": xf}


def _make_tiles():
    tiles = []
    o = 0
    while o < RC:
        R = min(RT, RC - o)
        tiles.append((o, R))
        o += R
    return tiles


def _chunks(total: int, step: int = MM):
    out = []
    s = 0
    while s < total:
        out.append((s, min(step, total - s)))
        s += step
    return out


# -------------------------------------------------------------- device side

@with_exitstack
def _energy_body(ctx: ExitStack, tc, out_d, out2_d, xh_d, xf_d, mask_d, bands_d, dbg=None):
    nc = tc.nc
    Exp = mybir.ActivationFunctionType.Exp
    Cpy = mybir.ActivationFunctionType.Copy
    rm_op = _register_recip_mul()
    Cpy2 = mybir.ActivationFunctionType.Copy

    # ---- constants. Conv bands are used directly as fp32r bitcast views;
    # BT/BB (ones bands) additionally as bf16 for the bf16 moving operands.
    consts = ctx.enter_context(tc.tile_pool(name="consts", bufs=1))
    scratch = consts.tile([1, 2], F32, name="scratch")
    nc.vector.memset(scratch, 0.0)
    # dummy activation at t=0 hoists the 1283ns Exp-table load off the
    # critical path (it would otherwise sit behind the first exp's waits)
    nc.scalar.activation(scratch[:, 0:1], scratch[:, 1:2],
                         mybir.ActivationFunctionType.Exp)
    warm = consts.tile([128, 128], F32R, name="warm")
    nc.vector.memset(warm.bitcast(F32), 0.0)
    bigb = consts.tile([128, 10 * 128], F32R, name="bigb")
    nc.sync.dma_start(
        out=bigb[:, 5 * 128 :].rearrange("p (i m) -> p i m", i=5),
        in_=bands_d[5:].rearrange("i p m -> p i m"),
    )

    def band(i):
        return bigb[:, i * 128 : (i + 1) * 128]

    MB = [band(v) for v in range(3)]       # conv bands, normal
    MBF = [band(5 + v) for v in range(3)]  # conv bands, folded
    BBb = consts.tile([128, 128], BF16, name="bbb")    # S band bf16, normal
    BTb = consts.tile([128, 128], BF16, name="btb")    # Z band bf16, normal
    BBFb = consts.tile([128, 128], BF16, name="bbfb")  # S band bf16, folded
    BTFb = consts.tile([128, 128], BF16, name="btfb")  # Z band bf16, folded
    nc.vector.tensor_copy(out=BBFb, in_=bigb[:, 8 * 128 : 9 * 128].bitcast(F32))
    nc.vector.tensor_copy(out=BTFb, in_=bigb[:, 9 * 128 : 10 * 128].bitcast(F32))

    def load_conv_bands():
        # deferred until after the first (folded) unit's X DMAs so the
        # pipeline-fill unit's inputs are first in the DMA queue
        nc.sync.dma_start(
            out=bigb[:, : 3 * 128].rearrange("p (i m) -> p i m", i=3),
            in_=bands_d[:3].rearrange("i p m -> p i m"),
        )

    def load_sum_bands():
        nc.sync.dma_start(
            out=bigb[:, 3 * 128 : 5 * 128].rearrange("p (i m) -> p i m", i=2),
            in_=bands_d[3:5].rearrange("i p m -> p i m"),
        )
        nc.vector.tensor_copy(out=BBb, in_=bigb[:, 3 * 128 : 4 * 128].bitcast(F32))
        nc.vector.tensor_copy(out=BTb, in_=bigb[:, 4 * 128 : 5 * 128].bitcast(F32))

    SEGW = WH // 4
    tiles = _make_tiles()
    RES_POOL_UNITS = {2, 4, 6, 8}

    # all row-tile exp-scale masks arrive in one small DMA (host pre-tiled)
    mk_all = consts.tile([128, len(tiles)], F32, name="mk_all")
    nc.sync.dma_start(out=mk_all, in_=mask_d)

    xpool = ctx.enter_context(tc.tile_pool(name="xp", bufs=7))
    epool = ctx.enter_context(tc.tile_pool(name="ep", bufs=7))
    tepool = ctx.enter_context(tc.tile_pool(name="tep", bufs=2))
    hepool = ctx.enter_context(tc.tile_pool(name="hep", bufs=3))
    upool = ctx.enter_context(tc.tile_pool(name="up", bufs=3))
    tupool = ctx.enter_context(tc.tile_pool(name="tup", bufs=3))
    sbpool = ctx.enter_context(tc.tile_pool(name="sbp", bufs=3))
    zspool = ctx.enter_context(tc.tile_pool(name="zsp", bufs=2))
    respool = ctx.enter_context(tc.tile_pool(name="resp", bufs=4))
    ps_conv = ctx.enter_context(tc.tile_pool(name="psc", bufs=2, space="PSUM"))
    ps_z = ctx.enter_context(tc.tile_pool(name="psz", bufs=2, space="PSUM"))
    ps_s = ctx.enter_context(tc.tile_pool(name="pss", bufs=2, space="PSUM"))

    _prep_count = [0]

    def prep(unit, q=None):
        """Allocate per-unit tiles + DMA X (called one iteration ahead).
        q overrides the DMA issue queue (Act for the first loads: overlaps
        SP's issue latency during fill; Act is idle until the first exp)."""
        o, R, g0, cw, fold, tj, z3 = unit
        q = q or nc.sync
        _prep_count[0] += 1
        EW = (SEGW if fold else cw) + 4   # E width
        UW = EW - 2                       # U / Rz width
        X = xpool.tile([128, EW + 2], F32R, tag="X")
        if fold:
            nc.sync.dma_start(out=X, in_=xf_d[g0 // WH])
        else:
            half = (cw + 6) // 2
            nc.sync.dma_start(
                out=X[: R + 6, :half], in_=xh_d[o : o + R + 6, g0 : g0 + half]
            )
            q.dma_start(
                out=X[: R + 6, half : cw + 6],
                in_=xh_d[o : o + R + 6, g0 + half : g0 + cw + 6],
            )
        return dict(
            o=o, R=R, g0=g0, cw=cw, fold=fold, tj=tj, z3=z3, EW=EW, UW=UW,
            res_pool=(not fold) and (tj != len(tiles) - 1)
            and (_prep_count[0] % 2 == 0),
            X=X,
            rows_in=slice(0, 128) if fold else slice(0, R + 6),
            rows_e=slice(0, 128) if fold else slice(0, R + 4),
            rows_s=slice(0, 128) if fold else slice(0, R + 2),
            OW=SEGW if fold else cw,
        )

    def emit_conv_chunk(st, cs, cl):
        """conv chunk (PE x3 per 512 sub-chunk, fp32r) into a 1024-wide
        (2-bank) PSUM tile -> one wide exp per tile (Act, bf16 out)."""
        rows_in, rows_e = st["rows_in"], st["rows_e"]
        mb = MBF if st["fold"] else MB
        mk = mk_all[:, st["tj"] : st["tj"] + 1]
        pc = ps_conv.tile([128, cl], F32, tag="pc", name="pc")
        for bs, bl in _chunks(cl):
            for v in range(3):
                nc.tensor.matmul(
                    pc[rows_e, bs : bs + bl],
                    mb[v][rows_in, rows_e],
                    st["X"][rows_in, cs + bs + v : cs + bs + v + bl],
                    start=(v == 0),
                    stop=(v == 2),
                )
        nc.scalar.activation(
            st["E"][rows_e, cs : cs + cl], pc[rows_e, :cl], Exp,
            scale=mk if st["fold"] else mk[: st["R"] + 4],
        )

    def stage_hsum(st):
        """Edge-pad memsets (must precede t_e) + Zh off-PE path:
        t_e = E + E(shift1) (DVE bf16 2x), hE = t_e + E(shift2) (GpSimd).
        z3 units only get the memsets."""
        rows_e, EW, UW, E = st["rows_e"], st["EW"], st["UW"], st["E"]
        # E at global-edge pad columns must be exp(0)=1: the conv window
        # at pad col -1 / W overlaps one real column, so it is NOT zero
        if st["g0"] == 0:
            er = slice(0, 32) if st["fold"] else rows_e
            nc.vector.memset(E[er, 0:2], 1.0)
        if st["g0"] + st["cw"] == W:
            er = slice(96, 128) if st["fold"] else rows_e
            nc.vector.memset(E[er, EW - 2 : EW], 1.0)
        if st["z3"]:
            return
        t_e = tepool.tile([128, EW - 1], BF16, tag="te")
        nc.vector.tensor_add(
            out=t_e[rows_e, :], in0=E[rows_e, : EW - 1], in1=E[rows_e, 1:EW]
        )
        hE = hepool.tile([128, UW], BF16, tag="hE")
        nc.gpsimd.tensor_add(
            out=hE[rows_e, :], in0=t_e[rows_e, :UW], in1=E[rows_e, 2:EW]
        )
        st["hE"] = hE

    def emit_z_chunk(st, cs, cl):
        """Z vertical pass chunk (PE, per 512-wide bank) into a 1024-wide
        (2-bank) PSUM tile -> ONE fused U chunk per tile (DVE custom op,
        Z straight from PSUM, bf16 out): halves the DVE PSUM-access
        per-instruction overhead. Consumed in-iteration: PSUM is 8 banks."""
        rows_e = st["rows_e"]
        bt = BTFb if st["fold"] else BTb
        pz = ps_z.tile([128, cl], F32, tag="pz", name="pz")
        for bs, bl in _chunks(cl):
            if st["z3"]:
                for v in range(3):
                    nc.tensor.matmul(
                        pz[rows_e, bs : bs + bl],
                        bt[rows_e, rows_e],
                        st["E"][rows_e, cs + bs + v : cs + bs + v + bl],
                        start=(v == 0),
                        stop=(v == 2),
                    )
            else:
                nc.tensor.matmul(
                    pz[rows_e, bs : bs + bl],
                    bt[rows_e, rows_e],
                    st["hE"][rows_e, cs + bs : cs + bs + bl],
                    start=True,
                    stop=True,
                )
        if USE_CUSTOM_U:
            nc.vector._custom_dve(
                rm_op,
                out=st["U"][rows_e, cs : cs + cl],
                in0=pz[rows_e, :cl],
                in1=st["X"].bitcast(F32)[rows_e, 2 + cs : 2 + cs + cl],
                s0=RQ_C0, s1=RQ_C1, imm2=RQ_C2,
            )
        else:
            Zs = st["Zs"]
            nc.scalar.activation(Zs[rows_e, cs : cs + cl], pz[rows_e, :cl], Cpy2)
            nc.vector.reciprocal_approx_fast(
                out=Zs[rows_e, cs : cs + cl], in_=Zs[rows_e, cs : cs + cl]
            )
            nc.gpsimd.tensor_mul(
                out=st["U"][rows_e, cs : cs + cl],
                in0=st["X"].bitcast(F32)[rows_e, 2 + cs : 2 + cs + cl],
                in1=Zs[rows_e, cs : cs + cl],
            )

    def stage_tu(st):
        """t_u = U + U(shift1), bf16. DVE (2x) for hE-path units; GpSimd
        for z3 units (whose Pool is otherwise idle) to unload the DVE."""
        rows_e, UW, U = st["rows_e"], st["UW"], st["U"]
        t_u = tupool.tile([128, UW - 1], BF16, tag="tu")
        eng = nc.vector
        eng.tensor_add(
            out=t_u[rows_e, :], in0=U[rows_e, : UW - 1], in1=U[rows_e, 1:UW]
        )
        st["t_u"] = t_u

    def emit_s_chunk(st, cs, cl):
        """S chunk = BB@t_u + BB@U(shift2) (PE bf16) -> Act drains the
        S PSUM chunk to bf16 SBUF."""
        rows_e, rows_s = st["rows_e"], st["rows_s"]
        bb = BBFb if st["fold"] else BBb
        ps = ps_s.tile([128, MM], F32, tag="ps", name="ps")
        nc.tensor.matmul(
            ps[rows_s, :cl], bb[rows_e, rows_s],
            st["t_u"][rows_e, cs : cs + cl],
            start=True, stop=False,
        )
        nc.tensor.matmul(
            ps[rows_s, :cl], bb[rows_e, rows_s],
            st["U"][rows_e, cs + 2 : cs + 2 + cl],
            start=False, stop=True,
        )
        nc.scalar.activation(st["Sb"][rows_s, cs : cs + cl], ps[rows_s, :cl], Cpy)

    def stage_res(st):
        """res = E * Sdrain (DVE), one wide op. The last row-tile writes
        f32 (its bf16 store corrupts on HW); folds go per-32-block so each
        block's store can issue while the next block's res computes."""
        rows_s, OW = st["rows_s"], st["OW"]
        lastt = st["tj"] == len(tiles) - 1
        res = respool.tile(
            [128, OW], F32 if lastt else BF16,
            tag="resf" if lastt else "res", name="res",
        )
        if st["fold"]:
            for b in range(4):
                rs = slice(32 * b, 32 * b + st["R"] + 4)
                eng = nc.gpsimd if b % 2 else nc.vector
                eng.tensor_mul(
                    out=res[rs, :OW],
                    in0=st["E"][rs, 2 : 2 + OW],
                    in1=st["Sb"][rs, :OW],
                )
        else:
            # alternating units multiply on GpSimd: Pool has slack and this
            # op is emitted after hE, so it never delays hE (PE's Zv input)
            eng = nc.gpsimd if st.get("res_pool") else nc.vector
            eng.tensor_mul(
                out=res[rows_s, :OW],
                in0=st["E"][rows_s, 2 : 2 + OW],
                in1=st["Sb"][rows_s, :OW],
            )
        st["res"] = res

    def store(st):
        # lagged well behind stage_res so SP's out-DMA issue rarely waits
        # on an unfinished res (which would head-of-line-block the next
        # X prefetch in the queue). The last row-tile goes to the separate
        # f32 out2 tensor: bf16 stores from the small-R tile corrupt even
        # columns on real HW (CoreSim clean); f32 stores never did.
        o, R, g0, fold, res = st["o"], st["R"], st["g0"], st["fold"], st["res"]
        lastt = st["tj"] == len(tiles) - 1
        dst = out2_d if lastt else out_d
        ro = o - tiles[-1][0] if lastt else o
        if fold:
            qs = [nc.sync, nc.sync, nc.sync, nc.sync]
            for b in range(4):
                qs[b].dma_start(
                    out=dst[ro : ro + R, g0 + b * SEGW : g0 + (b + 1) * SEGW],
                    in_=res[32 * b + 2 : 32 * b + 2 + R, :SEGW],
                )
        else:
            nc.sync.dma_start(
                out=dst[ro : ro + R, g0 : g0 + st["cw"]], in_=res[2 : R + 2, :st["cw"]]
            )

    of, Rf = tiles[-1]
    units = []
    if len(tiles) > 1 and Rf <= 26:
        # Both folded units lead: their X DMAs are tiny (32-row blocks) so
        # the engines saturate immediately while the serial DMA queue
        # streams the f32 X tiles of the wide units. Their f32 stores also
        # leave the tail to the two half-width units (short drain chains).
        # z3 flags: folds + every 4th normal unit use the 3-pass Z (PE)
        # instead of t_e/hE (DVE+Pool) -- balances PE vs DVE load.
        units.append((of, Rf, 0, WH, True, len(tiles) - 1, True))
        nrm = []
        for j, (o, R) in enumerate(tiles[:-1]):
            for h in range(WS):
                nrm.append((o, R, h * WH, WH, False, j))
        # first normal unit split in two: halves the X DMA the fill waits on
        o0, R0, g00, cw0, f0, j0 = nrm[0]
        units.append((o0, R0, g00, cw0 // 2, f0, j0, False))
        units.append((o0, R0, g00 + cw0 // 2, cw0 // 2, f0, j0, False))
        for i, (o, R, g0, cw, fold, j) in enumerate(nrm[1:-1]):
            units.append((o, R, g0, cw, fold, j, i % 4 == 1))
        # (res_pool flags are set on states in prep below)
        # split the trailing normal unit in two: at drain time only the
        # cheap fold remains to hide a unit's cross-engine chain
        o, R, g0, cw, fold, j = nrm[-1]
        units.append((o, R, g0, cw // 2, fold, j, False))
        units.append((o, R, g0 + cw // 2, cw // 2, fold, j, False))
        units.append((of, Rf, WH, WH, True, len(tiles) - 1, True))
    else:
        for j, (o, R) in enumerate(tiles):
            for h in range(WS):
                units.append((o, R, h * WH, WH, False, j, h == 0))
    n_real = len(units)

    states = [None] * len(units)
    n = len(units)

    def is_dummy(idx):
        return idx >= n_real
    # ~3.5us of dummy matmuls while the first DMAs land: the PE p-state
    # ramps to full clock only after 3us of continuous work, so the first
    # real conv then runs at 2.4GHz instead of half speed
    for _ in range(9):
        pw = ps_conv.tile([128, MM], F32, tag="pc")
        nc.tensor.matmul(pw[:, :128], warm, warm, start=True, stop=True)

    states[0] = prep(units[0])
    load_conv_bands()
    if n > 1:
        states[1] = prep(units[1])
    load_sum_bands()
    for i in range(n + 6):
        # X prefetch two iterations ahead of conv
        if i + 2 < n:
            states[i + 2] = prep(units[i + 2])
        st_c = states[i] if i < n else None
        st_h = states[i - 1] if 1 <= i <= n else None
        st_z = states[i - 3] if 3 <= i <= n + 2 else None
        st_s = states[i - 4] if 4 <= i <= n + 3 else None

        if st_c is not None:
            st_c["E"] = epool.tile([128, st_c["EW"]], BF16, tag="E", name="E")
        if st_z is not None:
            st_z["U"] = upool.tile([128, st_z["UW"]], BF16, tag="U", name="U")
            if not USE_CUSTOM_U:
                st_z["Zs"] = zspool.tile([128, st_z["UW"]], F32, tag="Zs", name="Zs")
        if st_s is not None:
            st_s["Sb"] = sbpool.tile([128, st_s["OW"]], BF16, tag="Sb", name="Sb")

        # DVE queue first: edge memsets + t_e of unit i-1 (their inputs
        # finished last iteration), then Pool's hE
        if st_h is not None:
            stage_hsum(st_h)

        # PE stream interleaved chunk-wise so Act/DVE consumers never park
        cc = _chunks(st_c["EW"]) if st_c is not None else []
        zc = _chunks(st_z["UW"], 1024) if st_z is not None else []
        sc = _chunks(st_s["OW"]) if st_s is not None else []
        for k in range(max(len(cc), len(zc), len(sc))):
            if k < len(cc):
                emit_conv_chunk(st_c, *cc[k])
            if k < len(zc):
                emit_z_chunk(st_z, *zc[k])
            if k < len(sc):
                emit_s_chunk(st_s, *sc[k])

        if st_z is not None:
            stage_tu(st_z)
        if 5 <= i <= n + 4:
            stage_res(states[i - 5])
        if 6 <= i <= n + 5:
            store(states[i - 6])

    if dbg is not None:
        st = states[n_real - 1]
        for key in dbg:
            if key == "ps":
                psf = respool.tile([128, MM], F32, name="psf", tag="psf")
                nc.scalar.activation(psf, st["ps_dbg"], Cpy)
                nc.sync.dma_start(out=dbg[key], in_=psf)
            else:
                src_t = st[{"E": "E", "U": "U", "Sb": "Sb", "res": "res", "tu": "t_u"}[key]]
                nc.sync.dma_start(out=dbg[key], in_=src_t)


_CACHE: dict = {}


def _build(dbg_mode=False):
    key = ("nc_dbg" if dbg_mode else "nc")
    if key in _CACHE:
        return _CACHE[key]
    nc = bacc.Bacc(
        "TRN2", target_bir_lowering=False, debug=False, num_devices=N_CORES
    )
    xh_d = nc.dram_tensor(
        "xh", (RC + 2 * HALO + 26, W + 2 * HALO), F32R, kind="ExternalInput"
    ).ap()
    xf_d = nc.dram_tensor(
        "xf", (WS, 128, WH // 4 + 6), F32R, kind="ExternalInput"
    ).ap()
    mask_d = nc.dram_tensor(
        "mask", (128, len(_make_tiles())), F32, kind="ExternalInput"
    ).ap()
    bands_d = nc.dram_tensor("bands", (10, 128, 128), F32R, kind="ExternalInput").ap()
    out_d = nc.dram_tensor("out", (RC, W), BF16, kind="ExternalOutput").ap()
    out2_d = nc.dram_tensor(
        "out2", (RC - _make_tiles()[-1][0], W), F32, kind="ExternalOutput"
    ).ap()
    dbg = None
    if dbg_mode:
        SEGW = WH // 4
        dbg = {
            "E": nc.dram_tensor("dbgE", (128, SEGW + 4), BF16, kind="ExternalOutput").ap(),
            "U": nc.dram_tensor("dbgU", (128, SEGW + 2), BF16, kind="ExternalOutput").ap(),
            "Sb": nc.dram_tensor("dbgSb", (128, SEGW), BF16, kind="ExternalOutput").ap(),
            "res": nc.dram_tensor("dbgres", (128, SEGW), BF16, kind="ExternalOutput").ap(),
            "tu": nc.dram_tensor("dbgtu", (128, SEGW + 1), BF16, kind="ExternalOutput").ap(),
        }
    with tile.TileContext(nc) as tc:
        _energy_body(tc, out_d, out2_d, xh_d, xf_d, mask_d, bands_d, dbg=dbg)
    nc.compile()
    _CACHE[key] = nc
    return nc


def kernel(shareable_energy: np.ndarray, kernel: np.ndarray, **_run_kw) -> np.ndarray:
    x = np.ascontiguousarray(np.asarray(shareable_energy, np.float32))
    k = np.asarray(kernel, np.float32)
    assert x.shape == (H, W), x.shape
    nc = _build()
    bands = _make_bands(k)
    in_maps = [_make_core_inputs(x, bands, core) for core in range(N_CORES)]
    r = run_bass_kernel_spmd(nc, in_maps, core_ids=list(range(N_CORES)), **_run_kw)
    o_last = _make_tiles()[-1][0]
    out = np.concatenate(
        [
            np.concatenate(
                [
                    np.asarray(res["out"]).astype(np.float32)[:o_last],
                    np.asarray(res["out2"]),
                ],
                axis=0,
            )
            for res in r.results
        ],
        axis=0,
    )
    if _run_kw:
        _CACHE["last_result"] = r
    return out


# revision 47
# speedup vs baseline: 1.0265x; 1.0021x over previous
"""Trainium2 Bass kernel for nn_EnergyDistributionCNN (3x3 conv -> unfold ->
softmax over patch -> weighted -> fold overlap-add), 8 NeuronCores.

Math (algebraically identical to the torch/jax reference):
    out = conv3x3(x, k)            cross-correlation, zero pad 1
    E   = exp(out)
    Z   = boxsum3x3(E padded with ONES)   (zero pads contribute exp(0)=1)
    U   = x / Z
    S   = boxsum3x3(U zero-padded)
    result = E * S

Sharding: row-block across 8 cores with a 3-row halo sliced on the host
(zero-filled at the global edges) -- no device-to-device communication.

All post-conv tensors are bf16 (host-measured error ~1.5e-2 max rel vs
the 2e-2 gate); conv stays fp32r (bf16 conv alone costs 1.8e-2). The
output is stored bf16 and upcast on the host, halving the out-DMA on the
serial DMA_ENGINES resource.

Engine split per width-half row-tile unit (~2050 cols):
  PE: conv as 3 shifted banded matmuls (fp32r); Z vertical band pass on
    hE (bf16, 1 pass) -- or, on z3-flagged units, 3 shifted BT passes on
    E directly (rebalances DVE->PE); S = 2 accumulating passes
    (BB@t_u + BB@U-shifted, bf16).
  Scalar: exp (masked via per-partition scale, bf16 out) and Copy
    (S PSUM -> bf16 SBUF drain). Both live in the same act table set.
  DVE: t_e = E + E(shift1) (bf16 2x), the fused custom op
    U = x * recip(Z) (quadratic-seed reciprocal + multiply in ONE
    8-stage DVE pass, reading Z straight from PSUM), t_u = U + U(shift1),
    and res = E * Sdrain (all-bf16 2x).
  GpSimd: hE = t_e + E(shift2) (the one wide op Pool can afford).

The custom DVE op RECIP_MUL_QUAD_ANT: 1/Z = bitcast(~Z) * p(t) with
t = Z*bitcast(~Z) in [-4.5, -4] (exponent-flip identity) and p a
degree-2 minimax fit of 1/t on that interval (rel err 5.1e-5), then * x.
Fits the 8-stage DVE ALU pipeline exactly; registered via the documented
dve_ops extension path.

Schedule: one emission iteration advances every unit's pipeline stage by
one. Within an iteration the PE stream interleaves chunk-wise
[conv_k(i), Zv_k(i-2), Smm_k(i-3)] so each cross-engine consumer (exp_k,
U_k, drain_k) finds its producer just-finished instead of parking; Z
PSUM chunks are consumed by the fused U op within the same iteration
(PSUM is only 8 banks). X tiles are DMA-prefetched one iteration ahead
so conv never parks on HBM; stores lag 5 iterations so SP's out-DMA
issue never head-of-line-blocks the X prefetch stream.
"""

from contextlib import ExitStack

import numpy as np

import concourse.bacc as bacc
import concourse.mybir as mybir
import concourse.tile as tile
from concourse._compat import with_exitstack
from concourse.bass_utils import run_bass_kernel_spmd

F32 = mybir.dt.float32
F32R = mybir.dt.float32r
BF16 = mybir.dt.bfloat16

H = 4096
W = 4096
N_CORES = 8
RC = H // N_CORES  # rows per core
HALO = 3
RT = 122   # output rows per row-tile (RT + 6 <= 128 partitions)
WS = 2     # width splits (SBUF capacity)
WH = W // WS
MM = 512   # matmul moving-operand max free size / one fp32 PSUM bank
USE_CUSTOM_U = True

# quadratic minimax fit of 1/t on t in [-4.5, -4] (rel err 5.1e-5)
RQ_C0 = -0.7071054765951768
RQ_C1 = -0.16652166157425166
RQ_C2 = -0.013060520969582767


# ----------------------------------------------------- custom DVE op (fused)

_RECIP_MUL = None


def _register_recip_mul():
    """U = in1 * (1/in0) in one DVE pass: exponent-flip seed + quadratic
    polish + multiply. Registered through the documented dve_ops extension
    path (OPS append + sub-opcode row); sha computed at registration."""
    global _RECIP_MUL
    if _RECIP_MUL is not None:
        return _RECIP_MUL
    from concourse import dve_ops
    from concourse.dve_spec import AluOp, Bin, Spec, Src0, Src1, C0, C1, C2, lower
    from concourse.dve_uop import DveOpSpec

    name = "RECIP_MUL_QUAD_ANT"
    if name in dve_ops._SUB_OPCODE_FOR_NAME:
        _RECIP_MUL = next(op for op in dve_ops.OPS if op.name == name)
        return _RECIP_MUL

    _not = Bin(AluOp.BITWISE_NOT, Src0, Src0)
    _t = Src0 * _not
    body = ((_t * C2 + C1) * _t + C0) * _not * Src1

    def ref(in0, in1, c0, c1, c2):
        z = np.ascontiguousarray(in0, np.float32)
        nx = (~z.view(np.int32)).view(np.float32)
        t = z * nx
        return ((t * c2 + c1) * t + c0) * nx * np.asarray(in1, np.float32)

    spec = Spec(body=body, reference=ref)
    row = max(dve_ops._SUB_OPCODE_FOR_NAME.values()) + 1
    assert row < 0x20, "custom-DVE row field overflow"
    dve_ops._SUB_OPCODE_FOR_NAME[name] = row
    shas = {}
    for ver in ("v3", "v4"):
        uops = lower(spec, ver=ver)
        shas[ver] = DveOpSpec(name=name, opcode=row, uops=uops, rd1_en=True).sha(ver)
    op = dve_ops.DveOp(name, spec, subdim=False, uops_sha=shas)
    dve_ops.OPS.append(op)
    dve_ops.CUSTOM_DVE_SPECS[name] = spec
    _RECIP_MUL = op
    return op


# ---------------------------------------------------------------- host side

def _make_bands(k: np.ndarray) -> np.ndarray:
    """bands[v][p, m] = k[p-m, v] (conv, v=0..2); bands[3] = BB ones with
    p-m in 0..2 (S matmul); bands[4] = BT ones with m-p in 0..2 (Z).
    bands[5..9]: same five patterns as 4x block-diagonal 32x32 blocks, for
    the column-folded last row-tile."""
    bands = np.zeros((10, 128, 128), np.float32)
    idx = np.arange(128)
    for d in range(3):
        p = idx[d:]
        m = idx[: 128 - d]
        for v in range(3):
            bands[v, p, m] = k[d, v]
        bands[3, p, m] = 1.0
        bands[4, m, p] = 1.0
    for i in range(5):
        blk = bands[i][:32, :32]
        for b in range(4):
            bands[5 + i][32 * b : 32 * b + 32, 32 * b : 32 * b + 32] = blk
    return bands


def _make_core_inputs(x: np.ndarray, bands: np.ndarray, core: int):
    r0 = core * RC
    lo, hi = r0 - HALO, r0 + RC + HALO
    # 26 extra zero rows let the folded last tile load full 32-row blocks
    xh = np.zeros((RC + 2 * HALO + 26, W + 2 * HALO), np.float32)
    s_lo, s_hi = max(lo, 0), min(hi, H)
    xh[s_lo - lo : s_hi - lo, HALO : HALO + W] = x[s_lo:s_hi]
    gl = np.arange(lo, hi)
    mask = ((gl >= 0) & (gl < H)).astype(np.float32)
    # fold-unit X tiles pre-packed: 4 column blocks stacked in partitions,
    # so each fold unit's X arrives in ONE DMA instead of four
    tiles0 = _make_tiles()
    of0, _Rf0 = tiles0[-1]
    SEGW = (W // WS) // 4
    xf = np.zeros((WS, 128, SEGW + 6), np.float32)
    for u in range(WS):
        for b in range(4):
            c0 = u * (W // WS) + b * SEGW
            xf[u, 32 * b : 32 * b + 32, :] = xh[of0 : of0 + 32, c0 : c0 + SEGW + 6]
    # pre-tiled per-row-tile mask: column j = exp-scale rows for tile j
    # (rows o+1 .. o+R+4); the fold tile's column is laid out in its
    # 4x32-partition block structure with zeros on the unused lanes.
    tiles = _make_tiles()
    mk = np.zeros((128, len(tiles)), np.float32)
    for j, (o, R) in enumerate(tiles[:-1]):
        mk[: R + 4, j] = mask[o + 1 : o + R + 5]
    of, Rf = tiles[-1]
    if Rf <= 26:
        for b in range(4):
            mk[32 * b : 32 * b + Rf + 4, len(tiles) - 1] = mask[of + 1 : of + Rf + 5]
    else:
        mk[: Rf + 4, len(tiles) - 1] = mask[of + 1 : of + Rf + 5]
    return {"xh": xh, "mask": mk, "bands": bands, "xf": xf}


def _make_tiles():
    tiles = []
    o = 0
    while o < RC:
        R = min(RT, RC - o)
        tiles.append((o, R))
        o += R
    return tiles


def _chunks(total: int, step: int = MM):
    out = []
    s = 0
    while s < total:
        out.append((s, min(step, total - s)))
        s += step
    return out


# -------------------------------------------------------------- device side

@with_exitstack
def _energy_body(ctx: ExitStack, tc, out_d, out2_d, xh_d, xf_d, mask_d, bands_d, dbg=None):
    nc = tc.nc
    Exp = mybir.ActivationFunctionType.Exp
    Cpy = mybir.ActivationFunctionType.Copy
    rm_op = _register_recip_mul()
    Cpy2 = mybir.ActivationFunctionType.Copy

    # ---- constants. Conv bands are used directly as fp32r bitcast views;
    # BT/BB (ones bands) additionally as bf16 for the bf16 moving operands.
    consts = ctx.enter_context(tc.tile_pool(name="consts", bufs=1))
    scratch = consts.tile([1, 2], F32, name="scratch")
    nc.vector.memset(scratch, 0.0)
    # dummy activation at t=0 hoists the 1283ns Exp-table load off the
    # critical path (it would otherwise sit behind the first exp's waits)
    nc.scalar.activation(scratch[:, 0:1], scratch[:, 1:2],
                         mybir.ActivationFunctionType.Exp)
    warm = consts.tile([128, 128], F32R, name="warm")
    nc.vector.memset(warm.bitcast(F32), 0.0)
    bigb = consts.tile([128, 10 * 128], F32R, name="bigb")
    nc.sync.dma_start(
        out=bigb[:, 5 * 128 :].rearrange("p (i m) -> p i m", i=5),
        in_=bands_d[5:].rearrange("i p m -> p i m"),
    )

    def band(i):
        return bigb[:, i * 128 : (i + 1) * 128]

    MB = [band(v) for v in range(3)]       # conv bands, normal
    MBF = [band(5 + v) for v in range(3)]  # conv bands, folded
    BBb = consts.tile([128, 128], BF16, name="bbb")    # S band bf16, normal
    BTb = consts.tile([128, 128], BF16, name="btb")    # Z band bf16, normal
    BBFb = consts.tile([128, 128], BF16, name="bbfb")  # S band bf16, folded
    BTFb = consts.tile([128, 128], BF16, name="btfb")  # Z band bf16, folded
    nc.vector.tensor_copy(out=BBFb, in_=bigb[:, 8 * 128 : 9 * 128].bitcast(F32))
    nc.vector.tensor_copy(out=BTFb, in_=bigb[:, 9 * 128 : 10 * 128].bitcast(F32))

    def load_conv_bands():
        # deferred until after the first (folded) unit's X DMAs so the
        # pipeline-fill unit's inputs are first in the DMA queue
        nc.sync.dma_start(
            out=bigb[:, : 3 * 128].rearrange("p (i m) -> p i m", i=3),
            in_=bands_d[:3].rearrange("i p m -> p i m"),
        )

    def load_sum_bands():
        nc.sync.dma_start(
            out=bigb[:, 3 * 128 : 5 * 128].rearrange("p (i m) -> p i m", i=2),
            in_=bands_d[3:5].rearrange("i p m -> p i m"),
        )
        nc.vector.tensor_copy(out=BBb, in_=bigb[:, 3 * 128 : 4 * 128].bitcast(F32))
        nc.vector.tensor_copy(out=BTb, in_=bigb[:, 4 * 128 : 5 * 128].bitcast(F32))

    SEGW = WH // 4
    tiles = _make_tiles()
    RES_POOL_UNITS = {2, 4, 6, 8}

    # all row-tile exp-scale masks arrive in one small DMA (host pre-tiled)
    mk_all = consts.tile([128, len(tiles)], F32, name="mk_all")
    nc.sync.dma_start(out=mk_all, in_=mask_d)

    xpool = ctx.enter_context(tc.tile_pool(name="xp", bufs=7))
    epool = ctx.enter_context(tc.tile_pool(name="ep", bufs=7))
    tepool = ctx.enter_context(tc.tile_pool(name="tep", bufs=2))
    hepool = ctx.enter_context(tc.tile_pool(name="hep", bufs=3))
    upool = ctx.enter_context(tc.tile_pool(name="up", bufs=3))
    tupool = ctx.enter_context(tc.tile_pool(name="tup", bufs=3))
    sbpool = ctx.enter_context(tc.tile_pool(name="sbp", bufs=3))
    zspool = ctx.enter_context(tc.tile_pool(name="zsp", bufs=2))
    respool = ctx.enter_context(tc.tile_pool(name="resp", bufs=4))
    ps_conv = ctx.enter_context(tc.tile_pool(name="psc", bufs=2, space="PSUM"))
    ps_z = ctx.enter_context(tc.tile_pool(name="psz", bufs=2, space="PSUM"))
    ps_s = ctx.enter_context(tc.tile_pool(name="pss", bufs=2, space="PSUM"))

    _prep_count = [0]

    def prep(unit, q=None):
        """Allocate per-unit tiles + DMA X (called one iteration ahead).
        q overrides the DMA issue queue (Act for the first loads: overlaps
        SP's issue latency during fill; Act is idle until the first exp)."""
        o, R, g0, cw, fold, tj, z3 = unit
        q = q or nc.sync
        _prep_count[0] += 1
        EW = (SEGW if fold else cw) + 4   # E width
        UW = EW - 2                       # U / Rz width
        X = xpool.tile([128, EW + 2], F32R, tag="X")
        if fold:
            if _prep_count[0] == 1:
                nc.sync.dma_start(out=X, in_=xf_d[g0 // WH])
            else:
                for b in range(4):
                    nc.sync.dma_start(
                        out=X[32 * b : 32 * b + 32, :],
                        in_=xh_d[o : o + 32, g0 + b * SEGW : g0 + b * SEGW + SEGW + 6],
                    )
        else:
            half = (cw + 6) // 2
            nc.sync.dma_start(
                out=X[: R + 6, :half], in_=xh_d[o : o + R + 6, g0 : g0 + half]
            )
            q.dma_start(
                out=X[: R + 6, half : cw + 6],
                in_=xh_d[o : o + R + 6, g0 + half : g0 + cw + 6],
            )
        return dict(
            o=o, R=R, g0=g0, cw=cw, fold=fold, tj=tj, z3=z3, EW=EW, UW=UW,
            res_pool=(not fold) and (tj != len(tiles) - 1)
            and (_prep_count[0] % 2 == 0),
            X=X,
            rows_in=slice(0, 128) if fold else slice(0, R + 6),
            rows_e=slice(0, 128) if fold else slice(0, R + 4),
            rows_s=slice(0, 128) if fold else slice(0, R + 2),
            OW=SEGW if fold else cw,
        )

    def emit_conv_chunk(st, cs, cl):
        """conv chunk (PE x3 per 512 sub-chunk, fp32r) into a 1024-wide
        (2-bank) PSUM tile -> one wide exp per tile (Act, bf16 out)."""
        rows_in, rows_e = st["rows_in"], st["rows_e"]
        mb = MBF if st["fold"] else MB
        mk = mk_all[:, st["tj"] : st["tj"] + 1]
        pc = ps_conv.tile([128, cl], F32, tag="pc", name="pc")
        for bs, bl in _chunks(cl):
            for v in range(3):
                nc.tensor.matmul(
                    pc[rows_e, bs : bs + bl],
                    mb[v][rows_in, rows_e],
                    st["X"][rows_in, cs + bs + v : cs + bs + v + bl],
                    start=(v == 0),
                    stop=(v == 2),
                )
        nc.scalar.activation(
            st["E"][rows_e, cs : cs + cl], pc[rows_e, :cl], Exp,
            scale=mk if st["fold"] else mk[: st["R"] + 4],
        )

    def stage_hsum(st):
        """Edge-pad memsets (must precede t_e) + Zh off-PE path:
        t_e = E + E(shift1) (DVE bf16 2x), hE = t_e + E(shift2) (GpSimd).
        z3 units only get the memsets."""
        rows_e, EW, UW, E = st["rows_e"], st["EW"], st["UW"], st["E"]
        # E at global-edge pad columns must be exp(0)=1: the conv window
        # at pad col -1 / W overlaps one real column, so it is NOT zero
        if st["g0"] == 0:
            er = slice(0, 32) if st["fold"] else rows_e
            nc.vector.memset(E[er, 0:2], 1.0)
        if st["g0"] + st["cw"] == W:
            er = slice(96, 128) if st["fold"] else rows_e
            nc.vector.memset(E[er, EW - 2 : EW], 1.0)
        if st["z3"]:
            return
        t_e = tepool.tile([128, EW - 1], BF16, tag="te")
        nc.vector.tensor_add(
            out=t_e[rows_e, :], in0=E[rows_e, : EW - 1], in1=E[rows_e, 1:EW]
        )
        hE = hepool.tile([128, UW], BF16, tag="hE")
        nc.gpsimd.tensor_add(
            out=hE[rows_e, :], in0=t_e[rows_e, :UW], in1=E[rows_e, 2:EW]
        )
        st["hE"] = hE

    def emit_z_chunk(st, cs, cl):
        """Z vertical pass chunk (PE, per 512-wide bank) into a 1024-wide
        (2-bank) PSUM tile -> ONE fused U chunk per tile (DVE custom op,
        Z straight from PSUM, bf16 out): halves the DVE PSUM-access
        per-instruction overhead. Consumed in-iteration: PSUM is 8 banks."""
        rows_e = st["rows_e"]
        bt = BTFb if st["fold"] else BTb
        pz = ps_z.tile([128, cl], F32, tag="pz", name="pz")
        for bs, bl in _chunks(cl):
            if st["z3"]:
                for v in range(3):
                    nc.tensor.matmul(
                        pz[rows_e, bs : bs + bl],
                        bt[rows_e, rows_e],
                        st["E"][rows_e, cs + bs + v : cs + bs + v + bl],
                        start=(v == 0),
                        stop=(v == 2),
                    )
            else:
                nc.tensor.matmul(
                    pz[rows_e, bs : bs + bl],
                    bt[rows_e, rows_e],
                    st["hE"][rows_e, cs + bs : cs + bs + bl],
                    start=True,
                    stop=True,
                )
        if USE_CUSTOM_U:
            nc.vector._custom_dve(
                rm_op,
                out=st["U"][rows_e, cs : cs + cl],
                in0=pz[rows_e, :cl],
                in1=st["X"].bitcast(F32)[rows_e, 2 + cs : 2 + cs + cl],
                s0=RQ_C0, s1=RQ_C1, imm2=RQ_C2,
            )
        else:
            Zs = st["Zs"]
            nc.scalar.activation(Zs[rows_e, cs : cs + cl], pz[rows_e, :cl], Cpy2)
            nc.vector.reciprocal_approx_fast(
                out=Zs[rows_e, cs : cs + cl], in_=Zs[rows_e, cs : cs + cl]
            )
            nc.gpsimd.tensor_mul(
                out=st["U"][rows_e, cs : cs + cl],
                in0=st["X"].bitcast(F32)[rows_e, 2 + cs : 2 + cs + cl],
                in1=Zs[rows_e, cs : cs + cl],
            )

    def stage_tu(st):
        """t_u = U + U(shift1), bf16. DVE (2x) for hE-path units; GpSimd
        for z3 units (whose Pool is otherwise idle) to unload the DVE."""
        rows_e, UW, U = st["rows_e"], st["UW"], st["U"]
        t_u = tupool.tile([128, UW - 1], BF16, tag="tu")
        eng = nc.vector
        eng.tensor_add(
            out=t_u[rows_e, :], in0=U[rows_e, : UW - 1], in1=U[rows_e, 1:UW]
        )
        st["t_u"] = t_u

    def emit_s_chunk(st, cs, cl):
        """S chunk = BB@t_u + BB@U(shift2) (PE bf16) -> Act drains the
        S PSUM chunk to bf16 SBUF."""
        rows_e, rows_s = st["rows_e"], st["rows_s"]
        bb = BBFb if st["fold"] else BBb
        ps = ps_s.tile([128, MM], F32, tag="ps", name="ps")
        nc.tensor.matmul(
            ps[rows_s, :cl], bb[rows_e, rows_s],
            st["t_u"][rows_e, cs : cs + cl],
            start=True, stop=False,
        )
        nc.tensor.matmul(
            ps[rows_s, :cl], bb[rows_e, rows_s],
            st["U"][rows_e, cs + 2 : cs + 2 + cl],
            start=False, stop=True,
        )
        nc.scalar.activation(st["Sb"][rows_s, cs : cs + cl], ps[rows_s, :cl], Cpy)

    def stage_res(st):
        """res = E * Sdrain (DVE), one wide op. The last row-tile writes
        f32 (its bf16 store corrupts on HW); folds go per-32-block so each
        block's store can issue while the next block's res computes."""
        rows_s, OW = st["rows_s"], st["OW"]
        lastt = st["tj"] == len(tiles) - 1
        res = respool.tile(
            [128, OW], F32 if lastt else BF16,
            tag="resf" if lastt else "res", name="res",
        )
        if st["fold"]:
            for b in range(4):
                rs = slice(32 * b, 32 * b + st["R"] + 4)
                eng = nc.gpsimd if b % 2 else nc.vector
                eng.tensor_mul(
                    out=res[rs, :OW],
                    in0=st["E"][rs, 2 : 2 + OW],
                    in1=st["Sb"][rs, :OW],
                )
        else:
            # alternating units multiply on GpSimd: Pool has slack and this
            # op is emitted after hE, so it never delays hE (PE's Zv input)
            eng = nc.gpsimd if st.get("res_pool") else nc.vector
            eng.tensor_mul(
                out=res[rows_s, :OW],
                in0=st["E"][rows_s, 2 : 2 + OW],
                in1=st["Sb"][rows_s, :OW],
            )
        st["res"] = res

    def store(st):
        # lagged well behind stage_res so SP's out-DMA issue rarely waits
        # on an unfinished res (which would head-of-line-block the next
        # X prefetch in the queue). The last row-tile goes to the separate
        # f32 out2 tensor: bf16 stores from the small-R tile corrupt even
        # columns on real HW (CoreSim clean); f32 stores never did.
        o, R, g0, fold, res = st["o"], st["R"], st["g0"], st["fold"], st["res"]
        lastt = st["tj"] == len(tiles) - 1
        dst = out2_d if lastt else out_d
        ro = o - tiles[-1][0] if lastt else o
        if fold:
            qs = [nc.sync, nc.sync, nc.sync, nc.sync]
            for b in range(4):
                qs[b].dma_start(
                    out=dst[ro : ro + R, g0 + b * SEGW : g0 + (b + 1) * SEGW],
                    in_=res[32 * b + 2 : 32 * b + 2 + R, :SEGW],
                )
        else:
            nc.sync.dma_start(
                out=dst[ro : ro + R, g0 : g0 + st["cw"]], in_=res[2 : R + 2, :st["cw"]]
            )

    of, Rf = tiles[-1]
    units = []
    if len(tiles) > 1 and Rf <= 26:
        # Both folded units lead: their X DMAs are tiny (32-row blocks) so
        # the engines saturate immediately while the serial DMA queue
        # streams the f32 X tiles of the wide units. Their f32 stores also
        # leave the tail to the two half-width units (short drain chains).
        # z3 flags: folds + every 4th normal unit use the 3-pass Z (PE)
        # instead of t_e/hE (DVE+Pool) -- balances PE vs DVE load.
        units.append((of, Rf, 0, WH, True, len(tiles) - 1, True))
        nrm = []
        for j, (o, R) in enumerate(tiles[:-1]):
            for h in range(WS):
                nrm.append((o, R, h * WH, WH, False, j))
        # first normal unit split in two: halves the X DMA the fill waits on
        o0, R0, g00, cw0, f0, j0 = nrm[0]
        units.append((o0, R0, g00, cw0 // 2, f0, j0, False))
        units.append((o0, R0, g00 + cw0 // 2, cw0 // 2, f0, j0, False))
        for i, (o, R, g0, cw, fold, j) in enumerate(nrm[1:-1]):
            units.append((o, R, g0, cw, fold, j, i % 4 == 1))
        # (res_pool flags are set on states in prep below)
        # split the trailing normal unit in two: at drain time only the
        # cheap fold remains to hide a unit's cross-engine chain
        o, R, g0, cw, fold, j = nrm[-1]
        units.append((o, R, g0, cw // 2, fold, j, False))
        units.append((o, R, g0 + cw // 2, cw // 2, fold, j, False))
        units.append((of, Rf, WH, WH, True, len(tiles) - 1, True))
    else:
        for j, (o, R) in enumerate(tiles):
            for h in range(WS):
                units.append((o, R, h * WH, WH, False, j, h == 0))
    n_real = len(units)

    states = [None] * len(units)
    n = len(units)

    def is_dummy(idx):
        return idx >= n_real
    # ~3.5us of dummy matmuls while the first DMAs land: the PE p-state
    # ramps to full clock only after 3us of continuous work, so the first
    # real conv then runs at 2.4GHz instead of half speed
    for _ in range(9):
        pw = ps_conv.tile([128, MM], F32, tag="pc")
        nc.tensor.matmul(pw[:, :128], warm, warm, start=True, stop=True)

    states[0] = prep(units[0])
    load_conv_bands()
    if n > 1:
        states[1] = prep(units[1])
    load_sum_bands()
    for i in range(n + 6):
        # X prefetch two iterations ahead of conv
        if i + 2 < n:
            states[i + 2] = prep(units[i + 2])
        st_c = states[i] if i < n else None
        st_h = states[i - 1] if 1 <= i <= n else None
        st_z = states[i - 3] if 3 <= i <= n + 2 else None
        st_s = states[i - 4] if 4 <= i <= n + 3 else None

        if st_c is not None:
            st_c["E"] = epool.tile([128, st_c["EW"]], BF16, tag="E", name="E")
        if st_z is not None:
            st_z["U"] = upool.tile([128, st_z["UW"]], BF16, tag="U", name="U")
            if not USE_CUSTOM_U:
                st_z["Zs"] = zspool.tile([128, st_z["UW"]], F32, tag="Zs", name="Zs")
        if st_s is not None:
            st_s["Sb"] = sbpool.tile([128, st_s["OW"]], BF16, tag="Sb", name="Sb")

        # DVE queue first: edge memsets + t_e of unit i-1 (their inputs
        # finished last iteration), then Pool's hE
        if st_h is not None:
            stage_hsum(st_h)

        # PE stream interleaved chunk-wise so Act/DVE consumers never park
        cc = _chunks(st_c["EW"]) if st_c is not None else []
        zc = _chunks(st_z["UW"], 1024) if st_z is not None else []
        sc = _chunks(st_s["OW"]) if st_s is not None else []
        for k in range(max(len(cc), len(zc), len(sc))):
            if k < len(cc):
                emit_conv_chunk(st_c, *cc[k])
            if k < len(zc):
                emit_z_chunk(st_z, *zc[k])
            if k < len(sc):
                emit_s_chunk(st_s, *sc[k])

        if st_z is not None:
            stage_tu(st_z)
        if 5 <= i <= n + 4:
            stage_res(states[i - 5])
        if 6 <= i <= n + 5:
            store(states[i - 6])

    if dbg is not None:
        st = states[n_real - 1]
        for key in dbg:
            if key == "ps":
                psf = respool.tile([128, MM], F32, name="psf", tag="psf")
                nc.scalar.activation(psf, st["ps_dbg"], Cpy)
                nc.sync.dma_start(out=dbg[key], in_=psf)
            else:
                src_t = st[{"E": "E", "U": "U", "Sb": "Sb", "res": "res", "tu": "t_u"}[key]]
                nc.sync.dma_start(out=dbg[key], in_=src_t)


_CACHE: dict = {}


def _build(dbg_mode=False):
    key = ("nc_dbg" if dbg_mode else "nc")
    if key in _CACHE:
        return _CACHE[key]
    nc = bacc.Bacc(
        "TRN2", target_bir_lowering=False, debug=False, num_devices=N_CORES
    )
    xh_d = nc.dram_tensor(
        "xh", (RC + 2 * HALO + 26, W + 2 * HALO), F32R, kind="ExternalInput"
    ).ap()
    xf_d = nc.dram_tensor(
        "xf", (WS, 128, WH // 4 + 6), F32R, kind="ExternalInput"
    ).ap()
    mask_d = nc.dram_tensor(
        "mask", (128, len(_make_tiles())), F32, kind="ExternalInput"
    ).ap()
    bands_d = nc.dram_tensor("bands", (10, 128, 128), F32R, kind="ExternalInput").ap()
    out_d = nc.dram_tensor("out", (RC, W), BF16, kind="ExternalOutput").ap()
    out2_d = nc.dram_tensor(
        "out2", (RC - _make_tiles()[-1][0], W), F32, kind="ExternalOutput"
    ).ap()
    dbg = None
    if dbg_mode:
        SEGW = WH // 4
        dbg = {
            "E": nc.dram_tensor("dbgE", (128, SEGW + 4), BF16, kind="ExternalOutput").ap(),
            "U": nc.dram_tensor("dbgU", (128, SEGW + 2), BF16, kind="ExternalOutput").ap(),
            "Sb": nc.dram_tensor("dbgSb", (128, SEGW), BF16, kind="ExternalOutput").ap(),
            "res": nc.dram_tensor("dbgres", (128, SEGW), BF16, kind="ExternalOutput").ap(),
            "tu": nc.dram_tensor("dbgtu", (128, SEGW + 1), BF16, kind="ExternalOutput").ap(),
        }
    with tile.TileContext(nc) as tc:
        _energy_body(tc, out_d, out2_d, xh_d, xf_d, mask_d, bands_d, dbg=dbg)
    nc.compile()
    _CACHE[key] = nc
    return nc


def kernel(shareable_energy: np.ndarray, kernel: np.ndarray, **_run_kw) -> np.ndarray:
    x = np.ascontiguousarray(np.asarray(shareable_energy, np.float32))
    k = np.asarray(kernel, np.float32)
    assert x.shape == (H, W), x.shape
    nc = _build()
    bands = _make_bands(k)
    in_maps = [_make_core_inputs(x, bands, core) for core in range(N_CORES)]
    r = run_bass_kernel_spmd(nc, in_maps, core_ids=list(range(N_CORES)), **_run_kw)
    o_last = _make_tiles()[-1][0]
    out = np.concatenate(
        [
            np.concatenate(
                [
                    np.asarray(res["out"]).astype(np.float32)[:o_last],
                    np.asarray(res["out2"]),
                ],
                axis=0,
            )
            for res in r.results
        ],
        axis=0,
    )
    if _run_kw:
        _CACHE["last_result"] = r
    return out


# revision 48
# speedup vs baseline: 1.0494x; 1.0223x over previous
"""Trainium2 Bass kernel for nn_EnergyDistributionCNN (3x3 conv -> unfold ->
softmax over patch -> weighted -> fold overlap-add), 8 NeuronCores.

Math (algebraically identical to the torch/jax reference):
    out = conv3x3(x, k)            cross-correlation, zero pad 1
    E   = exp(out)
    Z   = boxsum3x3(E padded with ONES)   (zero pads contribute exp(0)=1)
    U   = x / Z
    S   = boxsum3x3(U zero-padded)
    result = E * S

Sharding: row-block across 8 cores with a 3-row halo sliced on the host
(zero-filled at the global edges) -- no device-to-device communication.

All post-conv tensors are bf16 (host-measured error ~1.5e-2 max rel vs
the 2e-2 gate); conv stays fp32r (bf16 conv alone costs 1.8e-2). The
output is stored bf16 and upcast on the host, halving the out-DMA on the
serial DMA_ENGINES resource.

Engine split per width-half row-tile unit (~2050 cols):
  PE: conv as 3 shifted banded matmuls (fp32r); Z vertical band pass on
    hE (bf16, 1 pass) -- or, on z3-flagged units, 3 shifted BT passes on
    E directly (rebalances DVE->PE); S = 2 accumulating passes
    (BB@t_u + BB@U-shifted, bf16).
  Scalar: exp (masked via per-partition scale, bf16 out) and Copy
    (S PSUM -> bf16 SBUF drain). Both live in the same act table set.
  DVE: t_e = E + E(shift1) (bf16 2x), the fused custom op
    U = x * recip(Z) (quadratic-seed reciprocal + multiply in ONE
    8-stage DVE pass, reading Z straight from PSUM), t_u = U + U(shift1),
    and res = E * Sdrain (all-bf16 2x).
  GpSimd: hE = t_e + E(shift2) (the one wide op Pool can afford).

The custom DVE op RECIP_MUL_QUAD_ANT: 1/Z = bitcast(~Z) * p(t) with
t = Z*bitcast(~Z) in [-4.5, -4] (exponent-flip identity) and p a
degree-2 minimax fit of 1/t on that interval (rel err 5.1e-5), then * x.
Fits the 8-stage DVE ALU pipeline exactly; registered via the documented
dve_ops extension path.

Schedule: one emission iteration advances every unit's pipeline stage by
one. Within an iteration the PE stream interleaves chunk-wise
[conv_k(i), Zv_k(i-2), Smm_k(i-3)] so each cross-engine consumer (exp_k,
U_k, drain_k) finds its producer just-finished instead of parking; Z
PSUM chunks are consumed by the fused U op within the same iteration
(PSUM is only 8 banks). X tiles are DMA-prefetched one iteration ahead
so conv never parks on HBM; stores lag 5 iterations so SP's out-DMA
issue never head-of-line-blocks the X prefetch stream.
"""

from contextlib import ExitStack

import numpy as np

import concourse.bacc as bacc
import concourse.mybir as mybir
import concourse.tile as tile
from concourse._compat import with_exitstack
from concourse.bass_utils import run_bass_kernel_spmd

F32 = mybir.dt.float32
F32R = mybir.dt.float32r
BF16 = mybir.dt.bfloat16

H = 4096
W = 4096
N_CORES = 8
RC = H // N_CORES  # rows per core
HALO = 3
RT = 122   # output rows per row-tile (RT + 6 <= 128 partitions)
WS = 2     # width splits (SBUF capacity)
WH = W // WS
MM = 512   # matmul moving-operand max free size / one fp32 PSUM bank
USE_CUSTOM_U = True

# quadratic minimax fit of 1/t on t in [-4.5, -4] (rel err 5.1e-5)
RQ_C0 = -0.7071054765951768
RQ_C1 = -0.16652166157425166
RQ_C2 = -0.013060520969582767


# ----------------------------------------------------- custom DVE op (fused)

_RECIP_MUL = None


def _register_recip_mul():
    """U = in1 * (1/in0) in one DVE pass: exponent-flip seed + quadratic
    polish + multiply. Registered through the documented dve_ops extension
    path (OPS append + sub-opcode row); sha computed at registration."""
    global _RECIP_MUL
    if _RECIP_MUL is not None:
        return _RECIP_MUL
    from concourse import dve_ops
    from concourse.dve_spec import AluOp, Bin, Spec, Src0, Src1, C0, C1, C2, lower
    from concourse.dve_uop import DveOpSpec

    name = "RECIP_MUL_QUAD_ANT"
    if name in dve_ops._SUB_OPCODE_FOR_NAME:
        _RECIP_MUL = next(op for op in dve_ops.OPS if op.name == name)
        return _RECIP_MUL

    _not = Bin(AluOp.BITWISE_NOT, Src0, Src0)
    _t = Src0 * _not
    body = ((_t * C2 + C1) * _t + C0) * _not * Src1

    def ref(in0, in1, c0, c1, c2):
        z = np.ascontiguousarray(in0, np.float32)
        nx = (~z.view(np.int32)).view(np.float32)
        t = z * nx
        return ((t * c2 + c1) * t + c0) * nx * np.asarray(in1, np.float32)

    spec = Spec(body=body, reference=ref)
    row = max(dve_ops._SUB_OPCODE_FOR_NAME.values()) + 1
    assert row < 0x20, "custom-DVE row field overflow"
    dve_ops._SUB_OPCODE_FOR_NAME[name] = row
    shas = {}
    for ver in ("v3", "v4"):
        uops = lower(spec, ver=ver)
        shas[ver] = DveOpSpec(name=name, opcode=row, uops=uops, rd1_en=True).sha(ver)
    op = dve_ops.DveOp(name, spec, subdim=False, uops_sha=shas)
    dve_ops.OPS.append(op)
    dve_ops.CUSTOM_DVE_SPECS[name] = spec
    _RECIP_MUL = op
    return op


# ---------------------------------------------------------------- host side

def _make_bands(k: np.ndarray) -> np.ndarray:
    """bands[v][p, m] = k[p-m, v] (conv, v=0..2); bands[3] = BB ones with
    p-m in 0..2 (S matmul); bands[4] = BT ones with m-p in 0..2 (Z).
    bands[5..9]: same five patterns as 4x block-diagonal 32x32 blocks, for
    the column-folded last row-tile."""
    bands = np.zeros((10, 128, 128), np.float32)
    idx = np.arange(128)
    for d in range(3):
        p = idx[d:]
        m = idx[: 128 - d]
        for v in range(3):
            bands[v, p, m] = k[d, v]
        bands[3, p, m] = 1.0
        bands[4, m, p] = 1.0
    for i in range(5):
        blk = bands[i][:32, :32]
        for b in range(4):
            bands[5 + i][32 * b : 32 * b + 32, 32 * b : 32 * b + 32] = blk
    return bands


def _make_core_inputs(x: np.ndarray, bands: np.ndarray, core: int):
    r0 = core * RC
    lo, hi = r0 - HALO, r0 + RC + HALO
    # 26 extra zero rows let the folded last tile load full 32-row blocks
    xh = np.zeros((RC + 2 * HALO + 26, W + 2 * HALO), np.float32)
    s_lo, s_hi = max(lo, 0), min(hi, H)
    xh[s_lo - lo : s_hi - lo, HALO : HALO + W] = x[s_lo:s_hi]
    gl = np.arange(lo, hi)
    mask = ((gl >= 0) & (gl < H)).astype(np.float32)
    # fold-unit X tiles pre-packed: 4 column blocks stacked in partitions,
    # so each fold unit's X arrives in ONE DMA instead of four
    tiles0 = _make_tiles()
    of0, _Rf0 = tiles0[-1]
    SEGW = (W // WS) // 4
    xf = np.zeros((WS, 128, SEGW + 6), np.float32)
    for u in range(WS):
        for b in range(4):
            c0 = u * (W // WS) + b * SEGW
            xf[u, 32 * b : 32 * b + 32, :] = xh[of0 : of0 + 32, c0 : c0 + SEGW + 6]
    # pre-tiled per-row-tile mask: column j = exp-scale rows for tile j
    # (rows o+1 .. o+R+4); the fold tile's column is laid out in its
    # 4x32-partition block structure with zeros on the unused lanes.
    tiles = _make_tiles()
    mk = np.zeros((128, len(tiles)), np.float32)
    for j, (o, R) in enumerate(tiles[:-1]):
        mk[: R + 4, j] = mask[o + 1 : o + R + 5]
    of, Rf = tiles[-1]
    if Rf <= 26:
        for b in range(4):
            mk[32 * b : 32 * b + Rf + 4, len(tiles) - 1] = mask[of + 1 : of + Rf + 5]
    else:
        mk[: Rf + 4, len(tiles) - 1] = mask[of + 1 : of + Rf + 5]
    return {"xh": xh, "mask": mk, "bands": bands, "xf": xf}


def _make_tiles():
    tiles = []
    o = 0
    while o < RC:
        R = min(RT, RC - o)
        tiles.append((o, R))
        o += R
    return tiles


def _chunks(total: int, step: int = MM):
    out = []
    s = 0
    while s < total:
        out.append((s, min(step, total - s)))
        s += step
    return out


# -------------------------------------------------------------- device side

@with_exitstack
def _energy_body(ctx: ExitStack, tc, out_d, out2_d, xh_d, xf_d, mask_d, bands_d, dbg=None):
    nc = tc.nc
    Exp = mybir.ActivationFunctionType.Exp
    Cpy = mybir.ActivationFunctionType.Copy
    rm_op = _register_recip_mul()
    Cpy2 = mybir.ActivationFunctionType.Copy

    # ---- constants. Conv bands are used directly as fp32r bitcast views;
    # BT/BB (ones bands) additionally as bf16 for the bf16 moving operands.
    consts = ctx.enter_context(tc.tile_pool(name="consts", bufs=1))
    scratch = consts.tile([1, 2], F32, name="scratch")
    nc.vector.memset(scratch, 0.0)
    # dummy activation at t=0 hoists the 1283ns Exp-table load off the
    # critical path (it would otherwise sit behind the first exp's waits)
    nc.scalar.activation(scratch[:, 0:1], scratch[:, 1:2],
                         mybir.ActivationFunctionType.Exp)
    warm = consts.tile([128, 128], F32R, name="warm")
    nc.vector.memset(warm.bitcast(F32), 0.0)
    bigb = consts.tile([128, 10 * 128], F32R, name="bigb")
    nc.sync.dma_start(
        out=bigb[:, 5 * 128 :].rearrange("p (i m) -> p i m", i=5),
        in_=bands_d[5:].rearrange("i p m -> p i m"),
    )

    def band(i):
        return bigb[:, i * 128 : (i + 1) * 128]

    MB = [band(v) for v in range(3)]       # conv bands, normal
    MBF = [band(5 + v) for v in range(3)]  # conv bands, folded
    BBb = consts.tile([128, 128], BF16, name="bbb")    # S band bf16, normal
    BTb = consts.tile([128, 128], BF16, name="btb")    # Z band bf16, normal
    BBFb = consts.tile([128, 128], BF16, name="bbfb")  # S band bf16, folded
    BTFb = consts.tile([128, 128], BF16, name="btfb")  # Z band bf16, folded
    nc.vector.tensor_copy(out=BBFb, in_=bigb[:, 8 * 128 : 9 * 128].bitcast(F32))
    nc.vector.tensor_copy(out=BTFb, in_=bigb[:, 9 * 128 : 10 * 128].bitcast(F32))

    def load_conv_bands():
        # deferred until after the first (folded) unit's X DMAs so the
        # pipeline-fill unit's inputs are first in the DMA queue
        nc.sync.dma_start(
            out=bigb[:, : 3 * 128].rearrange("p (i m) -> p i m", i=3),
            in_=bands_d[:3].rearrange("i p m -> p i m"),
        )

    def load_sum_bands():
        nc.sync.dma_start(
            out=bigb[:, 3 * 128 : 5 * 128].rearrange("p (i m) -> p i m", i=2),
            in_=bands_d[3:5].rearrange("i p m -> p i m"),
        )
        nc.vector.tensor_copy(out=BBb, in_=bigb[:, 3 * 128 : 4 * 128].bitcast(F32))
        nc.vector.tensor_copy(out=BTb, in_=bigb[:, 4 * 128 : 5 * 128].bitcast(F32))

    SEGW = WH // 4
    tiles = _make_tiles()
    RES_POOL_UNITS = {2, 4, 6, 8}

    # all row-tile exp-scale masks arrive in one small DMA (host pre-tiled)
    mk_all = consts.tile([128, len(tiles)], F32, name="mk_all")
    nc.sync.dma_start(out=mk_all, in_=mask_d)

    xpool = ctx.enter_context(tc.tile_pool(name="xp", bufs=7))
    epool = ctx.enter_context(tc.tile_pool(name="ep", bufs=7))
    tepool = ctx.enter_context(tc.tile_pool(name="tep", bufs=2))
    hepool = ctx.enter_context(tc.tile_pool(name="hep", bufs=3))
    upool = ctx.enter_context(tc.tile_pool(name="up", bufs=3))
    tupool = ctx.enter_context(tc.tile_pool(name="tup", bufs=3))
    sbpool = ctx.enter_context(tc.tile_pool(name="sbp", bufs=3))
    zspool = ctx.enter_context(tc.tile_pool(name="zsp", bufs=2))
    respool = ctx.enter_context(tc.tile_pool(name="resp", bufs=4))
    ps_conv = ctx.enter_context(tc.tile_pool(name="psc", bufs=2, space="PSUM"))
    ps_z = ctx.enter_context(tc.tile_pool(name="psz", bufs=2, space="PSUM"))
    ps_s = ctx.enter_context(tc.tile_pool(name="pss", bufs=2, space="PSUM"))

    _prep_count = [0]

    def prep(unit, q=None):
        """Allocate per-unit tiles + DMA X (called one iteration ahead).
        q overrides the DMA issue queue (Act for the first loads: overlaps
        SP's issue latency during fill; Act is idle until the first exp)."""
        o, R, g0, cw, fold, tj, z3 = unit
        q = q or nc.sync
        _prep_count[0] += 1
        EW = (SEGW if fold else cw) + 4   # E width
        UW = EW - 2                       # U / Rz width
        X = xpool.tile([128, EW + 2], F32R, tag="X")
        if fold:
            for b in range(4):
                nc.sync.dma_start(
                    out=X[32 * b : 32 * b + 32, :],
                    in_=xh_d[o : o + 32, g0 + b * SEGW : g0 + b * SEGW + SEGW + 6],
                )
        else:
            half = (cw + 6) // 2
            nc.sync.dma_start(
                out=X[: R + 6, :half], in_=xh_d[o : o + R + 6, g0 : g0 + half]
            )
            q.dma_start(
                out=X[: R + 6, half : cw + 6],
                in_=xh_d[o : o + R + 6, g0 + half : g0 + cw + 6],
            )
        return dict(
            o=o, R=R, g0=g0, cw=cw, fold=fold, tj=tj, z3=z3, EW=EW, UW=UW,
            res_pool=(not fold) and (tj != len(tiles) - 1)
            and (_prep_count[0] % 2 == 0),
            X=X,
            rows_in=slice(0, 128) if fold else slice(0, R + 6),
            rows_e=slice(0, 128) if fold else slice(0, R + 4),
            rows_s=slice(0, 128) if fold else slice(0, R + 2),
            OW=SEGW if fold else cw,
        )

    def emit_conv_chunk(st, cs, cl):
        """conv chunk (PE x3 per 512 sub-chunk, fp32r) into a 1024-wide
        (2-bank) PSUM tile -> one wide exp per tile (Act, bf16 out)."""
        rows_in, rows_e = st["rows_in"], st["rows_e"]
        mb = MBF if st["fold"] else MB
        mk = mk_all[:, st["tj"] : st["tj"] + 1]
        pc = ps_conv.tile([128, cl], F32, tag="pc", name="pc")
        for bs, bl in _chunks(cl):
            for v in range(3):
                nc.tensor.matmul(
                    pc[rows_e, bs : bs + bl],
                    mb[v][rows_in, rows_e],
                    st["X"][rows_in, cs + bs + v : cs + bs + v + bl],
                    start=(v == 0),
                    stop=(v == 2),
                )
        nc.scalar.activation(
            st["E"][rows_e, cs : cs + cl], pc[rows_e, :cl], Exp,
            scale=mk if st["fold"] else mk[: st["R"] + 4],
        )

    def stage_hsum(st):
        """Edge-pad memsets (must precede t_e) + Zh off-PE path:
        t_e = E + E(shift1) (DVE bf16 2x), hE = t_e + E(shift2) (GpSimd).
        z3 units only get the memsets."""
        rows_e, EW, UW, E = st["rows_e"], st["EW"], st["UW"], st["E"]
        # E at global-edge pad columns must be exp(0)=1: the conv window
        # at pad col -1 / W overlaps one real column, so it is NOT zero
        if st["g0"] == 0:
            er = slice(0, 32) if st["fold"] else rows_e
            nc.vector.memset(E[er, 0:2], 1.0)
        if st["g0"] + st["cw"] == W:
            er = slice(96, 128) if st["fold"] else rows_e
            nc.vector.memset(E[er, EW - 2 : EW], 1.0)
        if st["z3"]:
            return
        t_e = tepool.tile([128, EW - 1], BF16, tag="te")
        nc.vector.tensor_add(
            out=t_e[rows_e, :], in0=E[rows_e, : EW - 1], in1=E[rows_e, 1:EW]
        )
        hE = hepool.tile([128, UW], BF16, tag="hE")
        nc.gpsimd.tensor_add(
            out=hE[rows_e, :], in0=t_e[rows_e, :UW], in1=E[rows_e, 2:EW]
        )
        st["hE"] = hE

    def emit_z_chunk(st, cs, cl):
        """Z vertical pass chunk (PE, per 512-wide bank) into a 1024-wide
        (2-bank) PSUM tile -> ONE fused U chunk per tile (DVE custom op,
        Z straight from PSUM, bf16 out): halves the DVE PSUM-access
        per-instruction overhead. Consumed in-iteration: PSUM is 8 banks."""
        rows_e = st["rows_e"]
        bt = BTFb if st["fold"] else BTb
        pz = ps_z.tile([128, cl], F32, tag="pz", name="pz")
        for bs, bl in _chunks(cl):
            if st["z3"]:
                for v in range(3):
                    nc.tensor.matmul(
                        pz[rows_e, bs : bs + bl],
                        bt[rows_e, rows_e],
                        st["E"][rows_e, cs + bs + v : cs + bs + v + bl],
                        start=(v == 0),
                        stop=(v == 2),
                    )
            else:
                nc.tensor.matmul(
                    pz[rows_e, bs : bs + bl],
                    bt[rows_e, rows_e],
                    st["hE"][rows_e, cs + bs : cs + bs + bl],
                    start=True,
                    stop=True,
                )
        if USE_CUSTOM_U:
            nc.vector._custom_dve(
                rm_op,
                out=st["U"][rows_e, cs : cs + cl],
                in0=pz[rows_e, :cl],
                in1=st["X"].bitcast(F32)[rows_e, 2 + cs : 2 + cs + cl],
                s0=RQ_C0, s1=RQ_C1, imm2=RQ_C2,
            )
        else:
            Zs = st["Zs"]
            nc.scalar.activation(Zs[rows_e, cs : cs + cl], pz[rows_e, :cl], Cpy2)
            nc.vector.reciprocal_approx_fast(
                out=Zs[rows_e, cs : cs + cl], in_=Zs[rows_e, cs : cs + cl]
            )
            nc.gpsimd.tensor_mul(
                out=st["U"][rows_e, cs : cs + cl],
                in0=st["X"].bitcast(F32)[rows_e, 2 + cs : 2 + cs + cl],
                in1=Zs[rows_e, cs : cs + cl],
            )

    def stage_tu(st):
        """t_u = U + U(shift1), bf16. DVE (2x) for hE-path units; GpSimd
        for z3 units (whose Pool is otherwise idle) to unload the DVE."""
        rows_e, UW, U = st["rows_e"], st["UW"], st["U"]
        t_u = tupool.tile([128, UW - 1], BF16, tag="tu")
        eng = nc.vector
        eng.tensor_add(
            out=t_u[rows_e, :], in0=U[rows_e, : UW - 1], in1=U[rows_e, 1:UW]
        )
        st["t_u"] = t_u

    def emit_s_chunk(st, cs, cl):
        """S chunk = BB@t_u + BB@U(shift2) (PE bf16) -> Act drains the
        S PSUM chunk to bf16 SBUF."""
        rows_e, rows_s = st["rows_e"], st["rows_s"]
        bb = BBFb if st["fold"] else BBb
        ps = ps_s.tile([128, MM], F32, tag="ps", name="ps")
        nc.tensor.matmul(
            ps[rows_s, :cl], bb[rows_e, rows_s],
            st["t_u"][rows_e, cs : cs + cl],
            start=True, stop=False,
        )
        nc.tensor.matmul(
            ps[rows_s, :cl], bb[rows_e, rows_s],
            st["U"][rows_e, cs + 2 : cs + 2 + cl],
            start=False, stop=True,
        )
        nc.scalar.activation(st["Sb"][rows_s, cs : cs + cl], ps[rows_s, :cl], Cpy)

    def stage_res(st):
        """res = E * Sdrain (DVE), one wide op. The last row-tile writes
        f32 (its bf16 store corrupts on HW); folds go per-32-block so each
        block's store can issue while the next block's res computes."""
        rows_s, OW = st["rows_s"], st["OW"]
        lastt = st["tj"] == len(tiles) - 1
        res = respool.tile(
            [128, OW], F32 if lastt else BF16,
            tag="resf" if lastt else "res", name="res",
        )
        if st["fold"]:
            for b in range(4):
                rs = slice(32 * b, 32 * b + st["R"] + 4)
                eng = nc.gpsimd if b % 2 else nc.vector
                eng.tensor_mul(
                    out=res[rs, :OW],
                    in0=st["E"][rs, 2 : 2 + OW],
                    in1=st["Sb"][rs, :OW],
                )
        else:
            # alternating units multiply on GpSimd: Pool has slack and this
            # op is emitted after hE, so it never delays hE (PE's Zv input)
            eng = nc.gpsimd if st.get("res_pool") else nc.vector
            eng.tensor_mul(
                out=res[rows_s, :OW],
                in0=st["E"][rows_s, 2 : 2 + OW],
                in1=st["Sb"][rows_s, :OW],
            )
        st["res"] = res

    def store(st):
        # lagged well behind stage_res so SP's out-DMA issue rarely waits
        # on an unfinished res (which would head-of-line-block the next
        # X prefetch in the queue). The last row-tile goes to the separate
        # f32 out2 tensor: bf16 stores from the small-R tile corrupt even
        # columns on real HW (CoreSim clean); f32 stores never did.
        o, R, g0, fold, res = st["o"], st["R"], st["g0"], st["fold"], st["res"]
        lastt = st["tj"] == len(tiles) - 1
        dst = out2_d if lastt else out_d
        ro = o - tiles[-1][0] if lastt else o
        if fold:
            qs = [nc.sync, nc.sync, nc.sync, nc.sync]
            for b in range(4):
                qs[b].dma_start(
                    out=dst[ro : ro + R, g0 + b * SEGW : g0 + (b + 1) * SEGW],
                    in_=res[32 * b + 2 : 32 * b + 2 + R, :SEGW],
                )
        else:
            nc.sync.dma_start(
                out=dst[ro : ro + R, g0 : g0 + st["cw"]], in_=res[2 : R + 2, :st["cw"]]
            )

    of, Rf = tiles[-1]
    units = []
    if len(tiles) > 1 and Rf <= 26:
        # Both folded units lead: their X DMAs are tiny (32-row blocks) so
        # the engines saturate immediately while the serial DMA queue
        # streams the f32 X tiles of the wide units. Their f32 stores also
        # leave the tail to the two half-width units (short drain chains).
        # z3 flags: folds + every 4th normal unit use the 3-pass Z (PE)
        # instead of t_e/hE (DVE+Pool) -- balances PE vs DVE load.
        units.append((of, Rf, 0, WH, True, len(tiles) - 1, True))
        nrm = []
        for j, (o, R) in enumerate(tiles[:-1]):
            for h in range(WS):
                nrm.append((o, R, h * WH, WH, False, j))
        # first normal unit split in two: halves the X DMA the fill waits on
        o0, R0, g00, cw0, f0, j0 = nrm[0]
        units.append((o0, R0, g00, cw0 // 2, f0, j0, False))
        units.append((o0, R0, g00 + cw0 // 2, cw0 // 2, f0, j0, False))
        for i, (o, R, g0, cw, fold, j) in enumerate(nrm[1:-1]):
            units.append((o, R, g0, cw, fold, j, i % 4 == 1))
        # (res_pool flags are set on states in prep below)
        # split the trailing normal unit in two: at drain time only the
        # cheap fold remains to hide a unit's cross-engine chain
        o, R, g0, cw, fold, j = nrm[-1]
        units.append((o, R, g0, cw // 2, fold, j, False))
        units.append((o, R, g0 + cw // 2, cw // 2, fold, j, False))
        units.append((of, Rf, WH, WH, True, len(tiles) - 1, True))
    else:
        for j, (o, R) in enumerate(tiles):
            for h in range(WS):
                units.append((o, R, h * WH, WH, False, j, h == 0))
    n_real = len(units)

    states = [None] * len(units)
    n = len(units)

    def is_dummy(idx):
        return idx >= n_real
    # ~3.5us of dummy matmuls while the first DMAs land: the PE p-state
    # ramps to full clock only after 3us of continuous work, so the first
    # real conv then runs at 2.4GHz instead of half speed
    for _ in range(9):
        pw = ps_conv.tile([128, MM], F32, tag="pc")
        nc.tensor.matmul(pw[:, :128], warm, warm, start=True, stop=True)

    states[0] = prep(units[0])
    load_conv_bands()
    if n > 1:
        states[1] = prep(units[1])
    load_sum_bands()
    for i in range(n + 6):
        # X prefetch two iterations ahead of conv
        if i + 2 < n:
            states[i + 2] = prep(units[i + 2])
        st_c = states[i] if i < n else None
        st_h = states[i - 1] if 1 <= i <= n else None
        st_z = states[i - 3] if 3 <= i <= n + 2 else None
        st_s = states[i - 4] if 4 <= i <= n + 3 else None

        if st_c is not None:
            st_c["E"] = epool.tile([128, st_c["EW"]], BF16, tag="E", name="E")
        if st_z is not None:
            st_z["U"] = upool.tile([128, st_z["UW"]], BF16, tag="U", name="U")
            if not USE_CUSTOM_U:
                st_z["Zs"] = zspool.tile([128, st_z["UW"]], F32, tag="Zs", name="Zs")
        if st_s is not None:
            st_s["Sb"] = sbpool.tile([128, st_s["OW"]], BF16, tag="Sb", name="Sb")

        # DVE queue first: edge memsets + t_e of unit i-1 (their inputs
        # finished last iteration), then Pool's hE
        if st_h is not None:
            stage_hsum(st_h)

        # PE stream interleaved chunk-wise so Act/DVE consumers never park
        cc = _chunks(st_c["EW"]) if st_c is not None else []
        zc = _chunks(st_z["UW"], 1024) if st_z is not None else []
        sc = _chunks(st_s["OW"]) if st_s is not None else []
        for k in range(max(len(cc), len(zc), len(sc))):
            if k < len(cc):
                emit_conv_chunk(st_c, *cc[k])
            if k < len(zc):
                emit_z_chunk(st_z, *zc[k])
            if k < len(sc):
                emit_s_chunk(st_s, *sc[k])

        if st_z is not None:
            stage_tu(st_z)
        if 5 <= i <= n + 4:
            stage_res(states[i - 5])
        if 6 <= i <= n + 5:
            store(states[i - 6])

    if dbg is not None:
        st = states[n_real - 1]
        for key in dbg:
            if key == "ps":
                psf = respool.tile([128, MM], F32, name="psf", tag="psf")
                nc.scalar.activation(psf, st["ps_dbg"], Cpy)
                nc.sync.dma_start(out=dbg[key], in_=psf)
            else:
                src_t = st[{"E": "E", "U": "U", "Sb": "Sb", "res": "res", "tu": "t_u"}[key]]
                nc.sync.dma_start(out=dbg[key], in_=src_t)


_CACHE: dict = {}


def _build(dbg_mode=False):
    key = ("nc_dbg" if dbg_mode else "nc")
    if key in _CACHE:
        return _CACHE[key]
    nc = bacc.Bacc(
        "TRN2", target_bir_lowering=False, debug=False, num_devices=N_CORES
    )
    xh_d = nc.dram_tensor(
        "xh", (RC + 2 * HALO + 26, W + 2 * HALO), F32R, kind="ExternalInput"
    ).ap()
    xf_d = nc.dram_tensor(
        "xf", (WS, 128, WH // 4 + 6), F32R, kind="ExternalInput"
    ).ap()
    mask_d = nc.dram_tensor(
        "mask", (128, len(_make_tiles())), F32, kind="ExternalInput"
    ).ap()
    bands_d = nc.dram_tensor("bands", (10, 128, 128), F32R, kind="ExternalInput").ap()
    out_d = nc.dram_tensor("out", (RC, W), BF16, kind="ExternalOutput").ap()
    out2_d = nc.dram_tensor(
        "out2", (RC - _make_tiles()[-1][0], W), F32, kind="ExternalOutput"
    ).ap()
    dbg = None
    if dbg_mode:
        SEGW = WH // 4
        dbg = {
            "E": nc.dram_tensor("dbgE", (128, SEGW + 4), BF16, kind="ExternalOutput").ap(),
            "U": nc.dram_tensor("dbgU", (128, SEGW + 2), BF16, kind="ExternalOutput").ap(),
            "Sb": nc.dram_tensor("dbgSb", (128, SEGW), BF16, kind="ExternalOutput").ap(),
            "res": nc.dram_tensor("dbgres", (128, SEGW), BF16, kind="ExternalOutput").ap(),
            "tu": nc.dram_tensor("dbgtu", (128, SEGW + 1), BF16, kind="ExternalOutput").ap(),
        }
    with tile.TileContext(nc) as tc:
        _energy_body(tc, out_d, out2_d, xh_d, xf_d, mask_d, bands_d, dbg=dbg)
    nc.compile()
    _CACHE[key] = nc
    return nc


def kernel(shareable_energy: np.ndarray, kernel: np.ndarray, **_run_kw) -> np.ndarray:
    x = np.ascontiguousarray(np.asarray(shareable_energy, np.float32))
    k = np.asarray(kernel, np.float32)
    assert x.shape == (H, W), x.shape
    nc = _build()
    bands = _make_bands(k)
    in_maps = [_make_core_inputs(x, bands, core) for core in range(N_CORES)]
    r = run_bass_kernel_spmd(nc, in_maps, core_ids=list(range(N_CORES)), **_run_kw)
    o_last = _make_tiles()[-1][0]
    out = np.concatenate(
        [
            np.concatenate(
                [
                    np.asarray(res["out"]).astype(np.float32)[:o_last],
                    np.asarray(res["out2"]),
                ],
                axis=0,
            )
            for res in r.results
        ],
        axis=0,
    )
    if _run_kw:
        _CACHE["last_result"] = r
    return out
